# revision 41
# baseline (speedup 1.0000x reference)
"""Trainium2 Bass kernel for nn_DisRNNCellNet (time-decayed LSTM + noisy-OR).

Data-parallel over 8 NeuronCores: bsize 4096 -> 512/core (4096 flat samples
per core, incl. the 8 nodules). Per core a 32-step LSTM (hid=64) runs with
features on SBUF partitions and samples on the free dim, batch split in two
halves of 2048 that share 128-partition-dense ACT/DVE ops:

  pif_h0 [128,2048] = (f,i) gate preacts of half0; pif_h1 = (i,f) of half1
  tg2    [128,2048] = g preacts: rows 0:64 half1, 64:128 half0 (M=64 MMs)
  poo    [128,2048] = o preacts: rows 0:64 half0, 64:128 half1
  c2     [128,2048] = cell state: rows 0:64 half0, 64:128 half1

  ACT (all dense):  sig(pif0) sig(pif1) tanh(tg2) sig(poo) tanh(c2)
  DVE: dc2=c2*dec2 | ig,fdc per half (bases matched) | add | h per half

The gate permutations exist so every 2-input DVE op sees equal input base
partitions (walrus checkSBSameStartPartition). Decay 1/log(e+dt) is host-
precomputed, host-replicated over 64 partitions.

The steady-state loop is ACT-bound (10 dense [128,1024] sigmoid/tanh ops
per step, ~97% ACT occupancy); op granularity is pinned by PSUM (4x 4KB
preact tags) and by the 2-chunk stagger that hides the per-chunk
DVE->tanh(c)->h recurrence latency. Wall-clock trims beyond that come from
the edges: all constants ship in one packed DMA (HWDGE issue is ~625ns
each), step 0 is specialized for h=c=0 (K=64 x-only matmuls, c2=i*g
directly, no state memsets, no dec[0] transfer), a warmup matmul starts the
PE clock ramp early, and the final step writes both halves' h densely into
one hfin tile (no next-step xh split needed). The FC runs TRANSPOSED:
nodule-strided h columns are the stationary operand and fc2 the 1-wide
moving operand, so z lands as [128 b-samples, 8 nodules] — the matmuls cost
~nothing (PE time tracks output width), the sigmoid is [128,8] instead of
[1,1024], the noisy-OR tree runs at free size 4/2/1, and one [128,4] DMA
ships all products (host transposes and applies the 1-k*t3 leak affine).
"""

import math

import ml_dtypes
import numpy as np

import concourse.bass as bass
import concourse.mybir as mybir
import concourse.tile as tile
from concourse.bass_utils import run_bass_kernel_spmd

BF16 = mybir.dt.bfloat16
F32 = mybir.dt.float32
AF = mybir.ActivationFunctionType

STEP, BSIZE, NNOD, DIM, HID = 32, 4096, 8, 64, 64
NCORES = 8
BL = (BSIZE // NCORES) * NNOD  # 4096 flat samples per core
HALF = BL // 2  # 2048
NB = HALF // 512  # 512-wide matmul chunks per half

LAST_RESULT = None


def _split_multiwaits(nc, max_waits=1):
    """walrus in this env rejects >1 sem wait per instruction ("Too many
    sync wait commands"); split extras onto single-wait NoOps."""
    for bb in nc.main_func.blocks:
        out = []
        for ins in bb.instructions:
            si = ins.sync_info
            if si is not None and len(si.on_wait) > max_waits:
                waits = list(si.on_wait)
                for j, w in enumerate(waits[:-max_waits]):
                    out.append(
                        mybir.InstNoOp(
                            name=f"{ins.name}-wsplit{j}",
                            engine=ins.engine,
                            ins=[],
                            outs=[],
                            sync_info=mybir.SyncInfo(on_wait=[w], on_update=[]),
                        )
                    )
                ins.sync_info = mybir.SyncInfo(
                    on_wait=waits[-max_waits:], on_update=list(si.on_update)
                )
            out.append(ins)
        bb.instructions = out


def _build(fc2_b: float, k_base: float):
    nc = bass.Bass(target_bir_lowering=False)
    x_d = nc.declare_dram_parameter("x", [STEP, DIM, BL], BF16, isOutput=False)
    dec_d = nc.declare_dram_parameter("dec", [STEP, 128, HALF], BF16, isOutput=False)
    # all bf16 weights packed in one buffer: wfi|wif|wg|wo|fc2(padded)
    wpack_d = nc.declare_dram_parameter("wpack", [128, 385], BF16, isOutput=False)
    # all f32 biases packed: bfi|bif|bg|bo columns
    bpack_d = nc.declare_dram_parameter("bpack", [128, 4], F32, isOutput=False)
    # noisy-OR products, [b-sample-within-block, block]: host transposes
    # to get core b-sample order; a single tile/DMA keeps the drain to one
    # HWDGE issue
    out_d = nc.declare_dram_parameter("out", [128, 4], F32, isOutput=True)

    with tile.TileContext(nc) as tc:
        with (
            tc.tile_pool(name="const", bufs=1) as const,
            tc.tile_pool(name="decp", bufs=2) as decp,
            tc.tile_pool(name="work", bufs=3) as work,
            tc.tile_pool(name="psum", bufs=1, space="PSUM") as psum,
        ):
            wpack = const.tile([128, 385], BF16, tag="wpack", name="wpack")
            bpack = const.tile([128, 4], F32, tag="bpack", name="bpack")
            nc.sync.dma_start(out=wpack[:], in_=wpack_d[:])
            wfi = wpack[:, 0:128]
            wif = wpack[:, 128:256]
            wg = wpack[:, 256:320]
            wo = wpack[:, 320:384]
            # fc2 replicated at partitions 64:128 so its base partition
            # matches the h rows read directly out of xh (rows 64:128)
            fc2 = wpack[HID:128, 384:385]
            bfi = bpack[:, 0:1]
            bif = bpack[:, 1:2]
            bg = bpack[:, 2:3]
            bo = bpack[:, 3:4]

            # persistent state: ping/pong xh per half, packed cell state.
            # step 0 runs h=c=0 specialized (K=64 x-only matmuls, c2 = i*g),
            # so no state memsets and no dec[0] transfer are needed.
            xh = [
                [
                    const.tile([128, HALF], BF16, tag=f"xh{q}{p}", name=f"xh{q}{p}")
                    for p in range(2)
                ]
                for q in range(2)
            ]
            c2 = const.tile([128, HALF], BF16, tag="c2", name="c2")
            # final-step h, both halves dense (rows 0:64 = half0, 64:128 =
            # half1); only the FC reads it
            hfin = const.tile([128, HALF], BF16, tag="hfin", name="hfin")

            # PE p-state warmup: one garbage matmul on a zeroed scratch tile
            # starts the tensor engine's clock ramp ~3us before the first
            # real matmul lands, so step 0 runs at full speed (the scratch
            # init rides the otherwise-idle ACT engine)
            warm = const.tile([128, 512], BF16, tag="warm", name="warm")
            nc.scalar.memzero(warm[:])
            pwarm = psum.tile([128, 512], F32, tag="pA0", name="pwarm")
            nc.tensor.matmul(
                pwarm[0:64, :], warm[:, 0:64], warm[:], start=True, stop=True
            )

            NCH = 2  # free-dim chunks per half (each with its own psum slots)
            CW = HALF // NCH
            NBC = CW // 512
            for t in range(STEP):
                par = t % 2
                x0, x1 = xh[0][par], xh[1][par]
                n0, n1 = xh[0][1 - par], xh[1][1 - par]
                if t == 0:
                    # chunk-granular priming so the first matmuls/acts start
                    # as soon as the first quarter of step-0 data lands;
                    # biases ship right after the first chunk's x (they are
                    # only needed once the first sigmoid runs)
                    for ch in range(NCH):
                        chs = bass.ds(ch * CW, CW)
                        nc.sync.dma_start(
                            out=x0[0:DIM, chs], in_=x_d[t, :, bass.ds(ch * CW, CW)]
                        )
                        nc.sync.dma_start(
                            out=x1[0:DIM, chs],
                            in_=x_d[t, :, bass.ds(HALF + ch * CW, CW)],
                        )
                        if ch == 0:
                            nc.sync.dma_start(out=bpack[:], in_=bpack_d[:])
                else:
                    decb = decp.tile([128, HALF], BF16, tag="decb", name="decb")
                    nc.sync.dma_start(out=decb[:], in_=dec_d[t])
                    nc.sync.dma_start(out=x0[0:DIM, :], in_=x_d[t, :, bass.ts(0, HALF)])
                    nc.sync.dma_start(out=x1[0:DIM, :], in_=x_d[t, :, bass.ts(1, HALF)])

                # step 0: h rows of xh are uninitialized; contract over x only
                KC = DIM if t == 0 else 128
                wfiK, wifK, wgK, woK = wfi[0:KC], wif[0:KC], wg[0:KC], wo[0:KC]

                for ch in range(NCH):
                    cs = bass.ds(ch * CW, CW)
                    pif0 = psum.tile([128, CW], F32, tag=f"pA{ch}", name="pif0")
                    for j in range(NBC):
                        js = bass.ds(ch * CW + j * 512, 512)
                        ps = bass.ts(j, 512)
                        nc.tensor.matmul(
                            pif0[:, ps], wfiK[:], x0[0:KC, js], start=True, stop=True
                        )
                    tg2 = psum.tile([128, CW], F32, tag=f"pB{ch}", name="tg2")
                    for j in range(NBC):
                        js = bass.ds(ch * CW + j * 512, 512)
                        ps = bass.ts(j, 512)
                        nc.tensor.matmul(
                            tg2[0:HID, ps], wgK[:], x1[0:KC, js], start=True, stop=True
                        )
                        nc.tensor.matmul(
                            tg2[HID:128, ps], wgK[:], x0[0:KC, js], start=True, stop=True
                        )
                    sif0 = work.tile([128, HALF], BF16, tag="sif0", name="sif0")
                    nc.scalar.activation(
                        sif0[:, cs], pif0[:], AF.Sigmoid, bias=bfi[:]
                    )

                    pif1 = psum.tile([128, CW], F32, tag=f"pA{ch}", name="pif1")
                    for j in range(NBC):
                        js = bass.ds(ch * CW + j * 512, 512)
                        ps = bass.ts(j, 512)
                        nc.tensor.matmul(
                            pif1[:, ps], wifK[:], x1[0:KC, js], start=True, stop=True
                        )
                    tgs = work.tile([128, HALF], BF16, tag="tgs", name="tgs")
                    nc.scalar.activation(tgs[:, cs], tg2[:], AF.Tanh, bias=bg[:])

                    poo = psum.tile([128, CW], F32, tag=f"pB{ch}", name="poo")
                    for j in range(NBC):
                        js = bass.ds(ch * CW + j * 512, 512)
                        ps = bass.ts(j, 512)
                        nc.tensor.matmul(
                            poo[0:HID, ps], woK[:], x0[0:KC, js], start=True, stop=True
                        )
                        nc.tensor.matmul(
                            poo[HID:128, ps], woK[:], x1[0:KC, js], start=True, stop=True
                        )
                    sif1 = work.tile([128, HALF], BF16, tag="sif1", name="sif1")
                    nc.scalar.activation(
                        sif1[:, cs], pif1[:], AF.Sigmoid, bias=bif[:]
                    )
                    so2 = work.tile([128, HALF], BF16, tag="so2", name="so2")
                    nc.scalar.activation(so2[:, cs], poo[:], AF.Sigmoid, bias=bo[:])

                    if t == 0:
                        # c0 = 0: cell state is just i*g, written straight
                        # into c2 (no decay/forget path this step)
                        nc.vector.tensor_mul(
                            c2[0:HID, cs], sif0[HID:128, cs], tgs[HID:128, cs]
                        )
                        nc.vector.tensor_mul(
                            c2[HID:128, cs], sif1[0:HID, cs], tgs[0:HID, cs]
                        )
                    else:
                        # DVE cell update (bases matched per op)
                        dc2 = work.tile([128, HALF], BF16, tag="dc2", name="dc2")
                        nc.gpsimd.tensor_mul(dc2[:, cs], c2[:, cs], decb[:, cs])
                        igT = work.tile([128, HALF], BF16, tag="igT", name="igT")
                        fdT = work.tile([128, HALF], BF16, tag="fdT", name="fdT")
                        # half0: i at rows 64:128 of sif0, g(h0) at 64:128 of tgs
                        nc.vector.tensor_mul(
                            igT[0:HID, cs], sif0[HID:128, cs], tgs[HID:128, cs]
                        )
                        # half1: i at rows 0:64 of sif1, g(h1) at 0:64 of tgs
                        nc.vector.tensor_mul(
                            igT[HID:128, cs], sif1[0:HID, cs], tgs[0:HID, cs]
                        )
                        # half0: f at rows 0:64 of sif0, dc at rows 0:64
                        nc.vector.tensor_mul(
                            fdT[0:HID, cs], sif0[0:HID, cs], dc2[0:HID, cs]
                        )
                        # half1: f at rows 64:128 of sif1, dc at rows 64:128
                        nc.vector.tensor_mul(
                            fdT[HID:128, cs], sif1[HID:128, cs], dc2[HID:128, cs]
                        )
                        nc.vector.tensor_add(c2[:, cs], igT[:, cs], fdT[:, cs])
                    tch = work.tile([128, HALF], BF16, tag="tch", name="tch")
                    nc.scalar.activation(tch[:, cs], c2[:, cs], AF.Tanh)
                    if t == STEP - 1:
                        # no next step: h feeds only the FC, so both halves'
                        # h land in one dense op in a dedicated tile
                        nc.vector.tensor_mul(
                            hfin[:, cs], so2[:, cs], tch[:, cs]
                        )
                    else:
                        nc.vector.tensor_mul(
                            n0[HID:128, cs], so2[0:HID, cs], tch[0:HID, cs]
                        )
                        nc.vector.tensor_mul(
                            n1[HID:128, cs], so2[HID:128, cs], tch[HID:128, cs]
                        )

            # ---- final: q = 1 - sigmoid(h@w + b), noisy-OR over nodules ----
            # h is read straight out of the final-parity xh tiles (rows 64:
            # 128); FC matmuls/sigmoids are emitted chunk0-cols first so they
            # overlap step-31 chunk1 compute, with 2 psum tags ping-ponged.
            fpar = STEP % 2
            nb2 = const.tile([128, 1], F32, tag="nb2", name="nb2")
            nc.vector.memset(nb2[:], -fc2_b)
            # transposed FC: per block of 128 b-samples, 8 matmuls use the
            # nodule-strided h columns as the STATIONARY operand and fc2 as
            # the 1-wide moving operand, landing z[b-sample, nodule] as
            # [128, 8] in PSUM. PE cost tracks output width (1), so the
            # matmuls are ~free; the sigmoid collapses to [128,8] (192ns vs
            # 1038) and the noisy-OR tree runs at free size 4/2/1.
            t3a = const.tile([128, 4], F32, tag="t3a", name="t3a")
            # ch-major emission: both halves' ch0 blocks only need chunk-0's
            # h (ready one tanh earlier), so they must precede the ch1
            # blocks in ACT program order or they stall behind them
            for ch in range(NCH):
                for q in range(2):
                    fc2q = wpack[q * HID : (q + 1) * HID, 384:385]
                    blk = q * NCH + ch
                    cs = bass.ds(ch * CW, CW)
                    hv = hfin[q * HID : (q + 1) * HID, cs].rearrange(
                        "p (b n) -> p n b", n=NNOD
                    )
                    pz = psum.tile(
                        [128, NNOD],
                        F32,
                        tag=["pA0", "pB0", "pA1", "pB1"][blk],
                        name="pz",
                    )
                    for n in range(NNOD):
                        nc.tensor.matmul(
                            pz[:, n : n + 1],
                            hv[:, n, :],
                            fc2q[:],
                            start=True,
                            stop=True,
                        )
                    qb = work.tile([128, NNOD], F32, tag="qb", name="qb")
                    nc.scalar.activation(
                        qb[:], pz[:], AF.Sigmoid, scale=-1.0, bias=nb2[:]
                    )
                    t1b = work.tile([128, 4], F32, tag="t1b", name="t1b")
                    nc.vector.tensor_mul(t1b[:], qb[:, 0:4], qb[:, 4:8])
                    t2b = work.tile([128, 2], F32, tag="t2b", name="t2b")
                    nc.vector.tensor_mul(t2b[:], t1b[:, 0:2], t1b[:, 2:4])
                    nc.vector.tensor_mul(
                        t3a[:, blk : blk + 1], t2b[:, 0:1], t2b[:, 1:2]
                    )
            # the final affine 1 - k_base*t3 is applied host-side
            nc.sync.dma_start(out=out_d[:], in_=t3a[:])

    _split_multiwaits(nc)
    return nc


def kernel(input, time_dis, w_ih, w_hh, b_ih, b_hh, fc2_w, fc2_b, baseline):
    input = np.asarray(input, dtype=np.float32)
    time_dis = np.asarray(time_dis, dtype=np.float32)
    w_ih = np.asarray(w_ih, dtype=np.float32)
    w_hh = np.asarray(w_hh, dtype=np.float32)
    b_ih = np.asarray(b_ih, dtype=np.float32)
    b_hh = np.asarray(b_hh, dtype=np.float32)
    fc2_w = np.asarray(fc2_w, dtype=np.float32)
    fc2_b = np.asarray(fc2_b, dtype=np.float32)
    baseline = np.asarray(baseline, dtype=np.float32)

    bf = ml_dtypes.bfloat16
    bper = BSIZE // NCORES  # 512

    # gates^T = W^T.T @ [x;h], W = [w_ih | w_hh]  [256, 128]
    W = np.concatenate([w_ih, w_hh], axis=1)  # [256, 128]
    lhsT = np.ascontiguousarray(W.T)  # [128, 256] cols: i(0:64) f g o
    li, lf = lhsT[:, 0:64], lhsT[:, 64:128]
    lg, lo = lhsT[:, 128:192], lhsT[:, 192:256]
    wfi = np.concatenate([lf, li], axis=1)
    wif = np.concatenate([li, lf], axis=1)
    fc2col = np.zeros((128, 1), dtype=np.float32)
    fc2col[0:HID, 0] = fc2_w.reshape(HID)
    fc2col[HID:128, 0] = fc2_w.reshape(HID)
    wpack = np.ascontiguousarray(
        np.concatenate([wfi, wif, lg, lo, fc2col], axis=1)
    ).astype(bf)  # [128, 385]
    bias = (b_ih + b_hh).astype(np.float32)
    bi, bfg = bias[0:64], bias[64:128]
    bgg, bog = bias[128:192], bias[192:256]
    bpack = np.ascontiguousarray(
        np.stack(
            [
                np.concatenate([bfg, bi]),
                np.concatenate([bi, bfg]),
                np.concatenate([bgg, bgg]),
                np.concatenate([bog, bog]),
            ],
            axis=1,
        )
    )  # [128, 4] f32
    k_base = float(1.0 - 1.0 / (1.0 + math.exp(-float(baseline[0]))))

    nc = _build(float(fc2_b[0]), k_base)

    in_maps = []
    for k in range(NCORES):
        bs = slice(k * bper, (k + 1) * bper)
        xs = input[:, bs].reshape(STEP, BL, DIM)
        xs = np.ascontiguousarray(xs.transpose(0, 2, 1)).astype(bf)  # [S,64,BL]
        td = time_dis[bs]  # [512, 32]
        td_bn = np.repeat(td.T, NNOD, axis=1)  # [32, 4096] sample-major
        td_used = np.concatenate([td_bn[:1], td_bn[:-1]], axis=0)
        dec = (1.0 / np.log(math.e + td_used)).astype(bf)  # [32, BL]
        # dec2[t, 0:64, j] = dec[t, j] (half0) ; dec2[t, 64:128, j] = dec[t, HALF+j]
        dec2 = np.empty((STEP, 128, HALF), dtype=bf)
        dec2[:, 0:HID, :] = dec[:, None, 0:HALF]
        dec2[:, HID:128, :] = dec[:, None, HALF:BL]
        in_maps.append(
            {
                "x": xs,
                "dec": dec2,
                "wpack": wpack,
                "bpack": bpack,
            }
        )

    res = None
    last_err = None
    for _attempt in range(3):
        try:
            res = run_bass_kernel_spmd(nc, in_maps, list(range(NCORES)))
            break
        except Exception as e:  # transient NRT device errors recover on retry
            last_err = e
    if res is None:
        raise last_err
    global LAST_RESULT
    LAST_RESULT = res
    out = np.concatenate(
        [np.asarray(res.results[k]["out"]).T.reshape(bper) for k in range(NCORES)]
    )
    # device ships t3 = prod_n q; the noisy-OR leak affine runs here
    return (1.0 - k_base * out).astype(np.float32)



# revision 43
# speedup vs baseline: 1.0014x; 1.0014x over previous
"""Trainium2 Bass kernel for nn_DisRNNCellNet (time-decayed LSTM + noisy-OR).

Data-parallel over 8 NeuronCores: bsize 4096 -> 512/core (4096 flat samples
per core, incl. the 8 nodules). Per core a 32-step LSTM (hid=64) runs with
features on SBUF partitions and samples on the free dim, batch split in two
halves of 2048 that share 128-partition-dense ACT/DVE ops:

  pif_h0 [128,2048] = (f,i) gate preacts of half0; pif_h1 = (i,f) of half1
  tg2    [128,2048] = g preacts: rows 0:64 half1, 64:128 half0 (M=64 MMs)
  poo    [128,2048] = o preacts: rows 0:64 half0, 64:128 half1
  c2     [128,2048] = cell state: rows 0:64 half0, 64:128 half1

  ACT (all dense):  sig(pif0) sig(pif1) tanh(tg2) sig(poo) tanh(c2)
  DVE: dc2=c2*dec2 | ig,fdc per half (bases matched) | add | h per half

The gate permutations exist so every 2-input DVE op sees equal input base
partitions (walrus checkSBSameStartPartition). Decay 1/log(e+dt) is host-
precomputed, host-replicated over 64 partitions.

The steady-state loop is ACT-bound (10 dense [128,1024] sigmoid/tanh ops
per step, ~97% ACT occupancy); op granularity is pinned by PSUM (4x 4KB
preact tags) and by the 2-chunk stagger that hides the per-chunk
DVE->tanh(c)->h recurrence latency. Wall-clock trims beyond that come from
the edges: all constants ship in one packed DMA (HWDGE issue is ~625ns
each), step 0 is specialized for h=c=0 (K=64 x-only matmuls, c2=i*g
directly, no state memsets, no dec[0] transfer), a warmup matmul starts the
PE clock ramp early, and the final step writes both halves' h densely into
one hfin tile (no next-step xh split needed). The FC runs TRANSPOSED:
nodule-strided h columns are the stationary operand and fc2 the 1-wide
moving operand, so z lands as [128 b-samples, 8 nodules] — the matmuls cost
~nothing (PE time tracks output width), the sigmoid is [128,8] instead of
[1,1024], the noisy-OR tree runs at free size 4/2/1, and one [128,4] DMA
ships all products (host transposes and applies the 1-k*t3 leak affine).
"""

import math

import ml_dtypes
import numpy as np

import concourse.bass as bass
import concourse.mybir as mybir
import concourse.tile as tile
from concourse.bass_utils import run_bass_kernel_spmd

BF16 = mybir.dt.bfloat16
F32 = mybir.dt.float32
AF = mybir.ActivationFunctionType

STEP, BSIZE, NNOD, DIM, HID = 32, 4096, 8, 64, 64
NCORES = 8
BL = (BSIZE // NCORES) * NNOD  # 4096 flat samples per core
HALF = BL // 2  # 2048
NB = HALF // 512  # 512-wide matmul chunks per half

LAST_RESULT = None


def _split_multiwaits(nc, max_waits=1):
    """walrus in this env rejects >1 sem wait per instruction ("Too many
    sync wait commands"); split extras onto single-wait NoOps."""
    for bb in nc.main_func.blocks:
        out = []
        for ins in bb.instructions:
            si = ins.sync_info
            if si is not None and len(si.on_wait) > max_waits:
                waits = list(si.on_wait)
                for j, w in enumerate(waits[:-max_waits]):
                    out.append(
                        mybir.InstNoOp(
                            name=f"{ins.name}-wsplit{j}",
                            engine=ins.engine,
                            ins=[],
                            outs=[],
                            sync_info=mybir.SyncInfo(on_wait=[w], on_update=[]),
                        )
                    )
                ins.sync_info = mybir.SyncInfo(
                    on_wait=waits[-max_waits:], on_update=list(si.on_update)
                )
            out.append(ins)
        bb.instructions = out


def _build(fc2_b: float, k_base: float):
    nc = bass.Bass(target_bir_lowering=False)
    x_d = nc.declare_dram_parameter("x", [STEP, DIM, BL], BF16, isOutput=False)
    dec_d = nc.declare_dram_parameter("dec", [STEP, 128, HALF], BF16, isOutput=False)
    # all bf16 weights packed in one buffer: wfi|wif|wg|wo|fc2(padded)
    wpack_d = nc.declare_dram_parameter("wpack", [128, 385], BF16, isOutput=False)
    # all f32 biases packed: bfi|bif|bg|bo columns
    bpack_d = nc.declare_dram_parameter("bpack", [128, 4], F32, isOutput=False)
    # noisy-OR products, [b-sample-within-block, block]: host transposes
    # to get core b-sample order; a single tile/DMA keeps the drain to one
    # HWDGE issue
    out_d = nc.declare_dram_parameter("out", [128, 4], F32, isOutput=True)

    with tile.TileContext(nc) as tc:
        with (
            tc.tile_pool(name="const", bufs=1) as const,
            tc.tile_pool(name="decp", bufs=2) as decp,
            tc.tile_pool(name="work", bufs=3) as work,
            tc.tile_pool(name="psum", bufs=1, space="PSUM") as psum,
        ):
            wpack = const.tile([128, 385], BF16, tag="wpack", name="wpack")
            bpack = const.tile([128, 4], F32, tag="bpack", name="bpack")
            nc.sync.dma_start(out=wpack[:], in_=wpack_d[:])
            wfi = wpack[:, 0:128]
            wif = wpack[:, 128:256]
            wg = wpack[:, 256:320]
            wo = wpack[:, 320:384]
            # fc2 replicated at partitions 64:128 so its base partition
            # matches the h rows read directly out of xh (rows 64:128)
            fc2 = wpack[HID:128, 384:385]
            bfi = bpack[:, 0:1]
            bif = bpack[:, 1:2]
            bg = bpack[:, 2:3]
            bo = bpack[:, 3:4]

            # persistent state: ping/pong xh per half, packed cell state.
            # step 0 runs h=c=0 specialized (K=64 x-only matmuls, c2 = i*g),
            # so no state memsets and no dec[0] transfer are needed.
            xh = [
                [
                    const.tile([128, HALF], BF16, tag=f"xh{q}{p}", name=f"xh{q}{p}")
                    for p in range(2)
                ]
                for q in range(2)
            ]
            c2 = const.tile([128, HALF], BF16, tag="c2", name="c2")
            # final-step h, both halves dense (rows 0:64 = half0, 64:128 =
            # half1); only the FC reads it
            hfin = const.tile([128, HALF], BF16, tag="hfin", name="hfin")

            # PE p-state warmup: one garbage matmul on a zeroed scratch tile
            # starts the tensor engine's clock ramp ~3us before the first
            # real matmul lands, so step 0 runs at full speed (the scratch
            # init rides the otherwise-idle ACT engine)
            warm = const.tile([128, 512], BF16, tag="warm", name="warm")
            nc.scalar.memzero(warm[:])
            pwarm = psum.tile([128, 512], F32, tag="pA0", name="pwarm")
            nc.tensor.matmul(
                pwarm[0:64, :], warm[:, 0:64], warm[:], start=True, stop=True
            )

            NCH = 2  # free-dim chunks per half (each with its own psum slots)
            CW = HALF // NCH
            NBC = CW // 512
            for t in range(STEP):
                par = t % 2
                x0, x1 = xh[0][par], xh[1][par]
                n0, n1 = xh[0][1 - par], xh[1][1 - par]
                if t == 0:
                    # chunk-granular priming so the first matmuls/acts start
                    # as soon as the first quarter of step-0 data lands;
                    # biases ship right after the first chunk's x (they are
                    # only needed once the first sigmoid runs)
                    for ch in range(NCH):
                        chs = bass.ds(ch * CW, CW)
                        nc.sync.dma_start(
                            out=x0[0:DIM, chs], in_=x_d[t, :, bass.ds(ch * CW, CW)]
                        )
                        nc.sync.dma_start(
                            out=x1[0:DIM, chs],
                            in_=x_d[t, :, bass.ds(HALF + ch * CW, CW)],
                        )
                        if ch == 0:
                            nc.sync.dma_start(out=bpack[:], in_=bpack_d[:])
                else:
                    decb = decp.tile([128, HALF], BF16, tag="decb", name="decb")
                    nc.sync.dma_start(out=decb[:], in_=dec_d[t])
                    nc.sync.dma_start(out=x0[0:DIM, :], in_=x_d[t, :, bass.ts(0, HALF)])
                    nc.sync.dma_start(out=x1[0:DIM, :], in_=x_d[t, :, bass.ts(1, HALF)])

                # step 0: h rows of xh are uninitialized; contract over x only
                KC = DIM if t == 0 else 128
                wfiK, wifK, wgK, woK = wfi[0:KC], wif[0:KC], wg[0:KC], wo[0:KC]

                for ch in range(NCH):
                    cs = bass.ds(ch * CW, CW)
                    pif0 = psum.tile([128, CW], F32, tag=f"pA{ch}", name="pif0")
                    for j in range(NBC):
                        js = bass.ds(ch * CW + j * 512, 512)
                        ps = bass.ts(j, 512)
                        nc.tensor.matmul(
                            pif0[:, ps], wfiK[:], x0[0:KC, js], start=True, stop=True
                        )
                    tg2 = psum.tile([128, CW], F32, tag=f"pB{ch}", name="tg2")
                    for j in range(NBC):
                        js = bass.ds(ch * CW + j * 512, 512)
                        ps = bass.ts(j, 512)
                        nc.tensor.matmul(
                            tg2[0:HID, ps], wgK[:], x1[0:KC, js], start=True, stop=True
                        )
                        nc.tensor.matmul(
                            tg2[HID:128, ps], wgK[:], x0[0:KC, js], start=True, stop=True
                        )
                    sif0 = work.tile([128, HALF], BF16, tag="sif0", name="sif0")
                    nc.scalar.activation(
                        sif0[:, cs], pif0[:], AF.Sigmoid, bias=bfi[:]
                    )

                    pif1 = psum.tile([128, CW], F32, tag=f"pA{ch}", name="pif1")
                    for j in range(NBC):
                        js = bass.ds(ch * CW + j * 512, 512)
                        ps = bass.ts(j, 512)
                        nc.tensor.matmul(
                            pif1[:, ps], wifK[:], x1[0:KC, js], start=True, stop=True
                        )
                    tgs = work.tile([128, HALF], BF16, tag="tgs", name="tgs")
                    nc.scalar.activation(tgs[:, cs], tg2[:], AF.Tanh, bias=bg[:])

                    poo = psum.tile([128, CW], F32, tag=f"pB{ch}", name="poo")
                    for j in range(NBC):
                        js = bass.ds(ch * CW + j * 512, 512)
                        ps = bass.ts(j, 512)
                        nc.tensor.matmul(
                            poo[0:HID, ps], woK[:], x0[0:KC, js], start=True, stop=True
                        )
                        nc.tensor.matmul(
                            poo[HID:128, ps], woK[:], x1[0:KC, js], start=True, stop=True
                        )
                    sif1 = work.tile([128, HALF], BF16, tag="sif1", name="sif1")
                    nc.scalar.activation(
                        sif1[:, cs], pif1[:], AF.Sigmoid, bias=bif[:]
                    )
                    so2 = work.tile([128, HALF], BF16, tag="so2", name="so2")
                    nc.scalar.activation(so2[:, cs], poo[:], AF.Sigmoid, bias=bo[:])

                    if t == 0:
                        # c0 = 0: cell state is just i*g, written straight
                        # into c2 (no decay/forget path this step)
                        nc.vector.tensor_mul(
                            c2[0:HID, cs], sif0[HID:128, cs], tgs[HID:128, cs]
                        )
                        nc.vector.tensor_mul(
                            c2[HID:128, cs], sif1[0:HID, cs], tgs[0:HID, cs]
                        )
                    else:
                        # DVE cell update (bases matched per op)
                        dc2 = work.tile([128, HALF], BF16, tag="dc2", name="dc2")
                        nc.gpsimd.tensor_mul(dc2[:, cs], c2[:, cs], decb[:, cs])
                        igT = work.tile([128, HALF], BF16, tag="igT", name="igT")
                        fdT = work.tile([128, HALF], BF16, tag="fdT", name="fdT")
                        # at the very last chunk of the last step the cell
                        # update runs 512-col-granular so tanh/hfin can chase
                        # the first half down the drain; everywhere else one
                        # dense pass per op. (i at rows 64:128 of sif0 /
                        # 0:64 of sif1 per the permuted gate packing.)
                        halves = (
                            [bass.ds(ch * CW, 512), bass.ds(ch * CW + 512, 512)]
                            if (t == STEP - 1 and ch == 1)
                            else [cs]
                        )
                        for hsl in halves:
                            nc.vector.tensor_mul(
                                igT[0:HID, hsl], sif0[HID:128, hsl], tgs[HID:128, hsl]
                            )
                            nc.vector.tensor_mul(
                                igT[HID:128, hsl], sif1[0:HID, hsl], tgs[0:HID, hsl]
                            )
                            nc.vector.tensor_mul(
                                fdT[0:HID, hsl], sif0[0:HID, hsl], dc2[0:HID, hsl]
                            )
                            nc.vector.tensor_mul(
                                fdT[HID:128, hsl], sif1[HID:128, hsl], dc2[HID:128, hsl]
                            )
                            nc.vector.tensor_add(c2[:, hsl], igT[:, hsl], fdT[:, hsl])
                    tch = work.tile([128, HALF], BF16, tag="tch", name="tch")
                    if t == STEP - 1 and ch == 1:
                        for hoff in (0, 512):
                            hs = bass.ds(ch * CW + hoff, 512)
                            nc.scalar.activation(tch[:, hs], c2[:, hs], AF.Tanh)
                    else:
                        nc.scalar.activation(tch[:, cs], c2[:, cs], AF.Tanh)
                    if t == STEP - 1:
                        # no next step: h feeds only the FC, so both halves'
                        # h land straight in the dedicated hfin tile
                        if ch == 1:
                            for hoff in (0, 512):
                                hs = bass.ds(ch * CW + hoff, 512)
                                nc.vector.tensor_mul(
                                    hfin[:, hs], so2[:, hs], tch[:, hs]
                                )
                        else:
                            nc.vector.tensor_mul(
                                hfin[:, cs], so2[:, cs], tch[:, cs]
                            )
                    else:
                        nc.vector.tensor_mul(
                            n0[HID:128, cs], so2[0:HID, cs], tch[0:HID, cs]
                        )
                        nc.vector.tensor_mul(
                            n1[HID:128, cs], so2[HID:128, cs], tch[HID:128, cs]
                        )

            # ---- final: q = 1 - sigmoid(h@w + b), noisy-OR over nodules ----
            # h is read straight out of the final-parity xh tiles (rows 64:
            # 128); FC matmuls/sigmoids are emitted chunk0-cols first so they
            # overlap step-31 chunk1 compute, with 2 psum tags ping-ponged.
            fpar = STEP % 2
            nb2 = const.tile([128, 1], F32, tag="nb2", name="nb2")
            nc.vector.memset(nb2[:], -fc2_b)
            # transposed FC: per block of 128 b-samples, 8 matmuls use the
            # nodule-strided h columns as the STATIONARY operand and fc2 as
            # the 1-wide moving operand, landing z[b-sample, nodule] as
            # [128, 8] in PSUM. PE cost tracks output width (1), so the
            # matmuls are ~free; the sigmoid collapses to [128,8] (192ns vs
            # 1038) and the noisy-OR tree runs at free size 4/2/1.
            t3a = const.tile([128, 4], F32, tag="t3a", name="t3a")
            # ch-major emission: both halves' ch0 blocks only need chunk-0's
            # h (ready one tanh earlier), so they must precede the ch1
            # blocks in ACT program order or they stall behind them
            for ch in range(NCH):
                for q in range(2):
                    fc2q = wpack[q * HID : (q + 1) * HID, 384:385]
                    blk = q * NCH + ch
                    cs = bass.ds(ch * CW, CW)
                    hv = hfin[q * HID : (q + 1) * HID, cs].rearrange(
                        "p (b n) -> p n b", n=NNOD
                    )
                    pz = psum.tile(
                        [128, NNOD],
                        F32,
                        tag=["pA0", "pB0", "pA1", "pB1"][blk],
                        name="pz",
                    )
                    for n in range(NNOD):
                        nc.tensor.matmul(
                            pz[:, n : n + 1],
                            hv[:, n, :],
                            fc2q[:],
                            start=True,
                            stop=True,
                        )
                    qb = work.tile([128, NNOD], F32, tag="qb", name="qb")
                    nc.scalar.activation(
                        qb[:], pz[:], AF.Sigmoid, scale=-1.0, bias=nb2[:]
                    )
                    t1b = work.tile([128, 4], F32, tag="t1b", name="t1b")
                    nc.vector.tensor_mul(t1b[:], qb[:, 0:4], qb[:, 4:8])
                    t2b = work.tile([128, 2], F32, tag="t2b", name="t2b")
                    nc.vector.tensor_mul(t2b[:], t1b[:, 0:2], t1b[:, 2:4])
                    nc.vector.tensor_mul(
                        t3a[:, blk : blk + 1], t2b[:, 0:1], t2b[:, 1:2]
                    )
            # the final affine 1 - k_base*t3 is applied host-side
            nc.sync.dma_start(out=out_d[:], in_=t3a[:])

    _split_multiwaits(nc)
    return nc


def kernel(input, time_dis, w_ih, w_hh, b_ih, b_hh, fc2_w, fc2_b, baseline):
    input = np.asarray(input, dtype=np.float32)
    time_dis = np.asarray(time_dis, dtype=np.float32)
    w_ih = np.asarray(w_ih, dtype=np.float32)
    w_hh = np.asarray(w_hh, dtype=np.float32)
    b_ih = np.asarray(b_ih, dtype=np.float32)
    b_hh = np.asarray(b_hh, dtype=np.float32)
    fc2_w = np.asarray(fc2_w, dtype=np.float32)
    fc2_b = np.asarray(fc2_b, dtype=np.float32)
    baseline = np.asarray(baseline, dtype=np.float32)

    bf = ml_dtypes.bfloat16
    bper = BSIZE // NCORES  # 512

    # gates^T = W^T.T @ [x;h], W = [w_ih | w_hh]  [256, 128]
    W = np.concatenate([w_ih, w_hh], axis=1)  # [256, 128]
    lhsT = np.ascontiguousarray(W.T)  # [128, 256] cols: i(0:64) f g o
    li, lf = lhsT[:, 0:64], lhsT[:, 64:128]
    lg, lo = lhsT[:, 128:192], lhsT[:, 192:256]
    wfi = np.concatenate([lf, li], axis=1)
    wif = np.concatenate([li, lf], axis=1)
    fc2col = np.zeros((128, 1), dtype=np.float32)
    fc2col[0:HID, 0] = fc2_w.reshape(HID)
    fc2col[HID:128, 0] = fc2_w.reshape(HID)
    wpack = np.ascontiguousarray(
        np.concatenate([wfi, wif, lg, lo, fc2col], axis=1)
    ).astype(bf)  # [128, 385]
    bias = (b_ih + b_hh).astype(np.float32)
    bi, bfg = bias[0:64], bias[64:128]
    bgg, bog = bias[128:192], bias[192:256]
    bpack = np.ascontiguousarray(
        np.stack(
            [
                np.concatenate([bfg, bi]),
                np.concatenate([bi, bfg]),
                np.concatenate([bgg, bgg]),
                np.concatenate([bog, bog]),
            ],
            axis=1,
        )
    )  # [128, 4] f32
    k_base = float(1.0 - 1.0 / (1.0 + math.exp(-float(baseline[0]))))

    nc = _build(float(fc2_b[0]), k_base)

    in_maps = []
    for k in range(NCORES):
        bs = slice(k * bper, (k + 1) * bper)
        xs = input[:, bs].reshape(STEP, BL, DIM)
        xs = np.ascontiguousarray(xs.transpose(0, 2, 1)).astype(bf)  # [S,64,BL]
        td = time_dis[bs]  # [512, 32]
        td_bn = np.repeat(td.T, NNOD, axis=1)  # [32, 4096] sample-major
        td_used = np.concatenate([td_bn[:1], td_bn[:-1]], axis=0)
        dec = (1.0 / np.log(math.e + td_used)).astype(bf)  # [32, BL]
        # dec2[t, 0:64, j] = dec[t, j] (half0) ; dec2[t, 64:128, j] = dec[t, HALF+j]
        dec2 = np.empty((STEP, 128, HALF), dtype=bf)
        dec2[:, 0:HID, :] = dec[:, None, 0:HALF]
        dec2[:, HID:128, :] = dec[:, None, HALF:BL]
        in_maps.append(
            {
                "x": xs,
                "dec": dec2,
                "wpack": wpack,
                "bpack": bpack,
            }
        )

    res = None
    last_err = None
    for _attempt in range(3):
        try:
            res = run_bass_kernel_spmd(nc, in_maps, list(range(NCORES)))
            break
        except Exception as e:  # transient NRT device errors recover on retry
            last_err = e
    if res is None:
        raise last_err
    global LAST_RESULT
    LAST_RESULT = res
    out = np.concatenate(
        [np.asarray(res.results[k]["out"]).T.reshape(bper) for k in range(NCORES)]
    )
    # device ships t3 = prod_n q; the noisy-OR leak affine runs here
    return (1.0 - k_base * out).astype(np.float32)



# revision 44
# speedup vs baseline: 1.0016x; 1.0002x over previous
"""Trainium2 Bass kernel for nn_DisRNNCellNet (time-decayed LSTM + noisy-OR).

Data-parallel over 8 NeuronCores: bsize 4096 -> 512/core (4096 flat samples
per core, incl. the 8 nodules). Per core a 32-step LSTM (hid=64) runs with
features on SBUF partitions and samples on the free dim, batch split in two
halves of 2048 that share 128-partition-dense ACT/DVE ops:

  pif_h0 [128,2048] = (f,i) gate preacts of half0; pif_h1 = (i,f) of half1
  tg2    [128,2048] = g preacts: rows 0:64 half1, 64:128 half0 (M=64 MMs)
  poo    [128,2048] = o preacts: rows 0:64 half0, 64:128 half1
  c2     [128,2048] = cell state: rows 0:64 half0, 64:128 half1

  ACT (all dense):  sig(pif0) sig(pif1) tanh(tg2) sig(poo) tanh(c2)
  DVE: dc2=c2*dec2 | ig,fdc per half (bases matched) | add | h per half

The gate permutations exist so every 2-input DVE op sees equal input base
partitions (walrus checkSBSameStartPartition). Decay 1/log(e+dt) is host-
precomputed, host-replicated over 64 partitions.

The steady-state loop is ACT-bound (10 dense [128,1024] sigmoid/tanh ops
per step, ~97% ACT occupancy); op granularity is pinned by PSUM (4x 4KB
preact tags) and by the 2-chunk stagger that hides the per-chunk
DVE->tanh(c)->h recurrence latency. Wall-clock trims beyond that come from
the edges: all constants ship in one packed DMA (HWDGE issue is ~625ns
each), step 0 is specialized for h=c=0 (K=64 x-only matmuls, c2=i*g
directly, no state memsets, no dec[0] transfer), a warmup matmul starts the
PE clock ramp early, and the final step writes both halves' h densely into
one hfin tile (no next-step xh split needed). The FC runs TRANSPOSED:
nodule-strided h columns are the stationary operand and fc2 the 1-wide
moving operand, so z lands as [128 b-samples, 8 nodules] — the matmuls cost
~nothing (PE time tracks output width), the sigmoid is [128,8] instead of
[1,1024], the noisy-OR tree runs at free size 4/2/1, and one [128,4] DMA
ships all products (host transposes and applies the 1-k*t3 leak affine).
"""

import math

import ml_dtypes
import numpy as np

import concourse.bass as bass
import concourse.mybir as mybir
import concourse.tile as tile
from concourse.bass_utils import run_bass_kernel_spmd

BF16 = mybir.dt.bfloat16
F32 = mybir.dt.float32
AF = mybir.ActivationFunctionType

STEP, BSIZE, NNOD, DIM, HID = 32, 4096, 8, 64, 64
NCORES = 8
BL = (BSIZE // NCORES) * NNOD  # 4096 flat samples per core
HALF = BL // 2  # 2048
NB = HALF // 512  # 512-wide matmul chunks per half

LAST_RESULT = None


def _split_multiwaits(nc, max_waits=1):
    """walrus in this env rejects >1 sem wait per instruction ("Too many
    sync wait commands"); split extras onto single-wait NoOps."""
    for bb in nc.main_func.blocks:
        out = []
        for ins in bb.instructions:
            si = ins.sync_info
            if si is not None and len(si.on_wait) > max_waits:
                waits = list(si.on_wait)
                for j, w in enumerate(waits[:-max_waits]):
                    out.append(
                        mybir.InstNoOp(
                            name=f"{ins.name}-wsplit{j}",
                            engine=ins.engine,
                            ins=[],
                            outs=[],
                            sync_info=mybir.SyncInfo(on_wait=[w], on_update=[]),
                        )
                    )
                ins.sync_info = mybir.SyncInfo(
                    on_wait=waits[-max_waits:], on_update=list(si.on_update)
                )
            out.append(ins)
        bb.instructions = out


def _build(fc2_b: float, k_base: float):
    nc = bass.Bass(target_bir_lowering=False)
    x_d = nc.declare_dram_parameter("x", [STEP, DIM, BL], BF16, isOutput=False)
    dec_d = nc.declare_dram_parameter("dec", [STEP, 128, HALF], BF16, isOutput=False)
    # all bf16 weights packed in one buffer: wfi|wif|wg|wo|fc2(padded)
    wpack_d = nc.declare_dram_parameter("wpack", [128, 385], BF16, isOutput=False)
    # all f32 biases packed: bfi|bif|bg|bo columns
    bpack_d = nc.declare_dram_parameter("bpack", [128, 4], F32, isOutput=False)
    # noisy-OR products, [b-sample-within-block, block]: host transposes
    # to get core b-sample order; a single tile/DMA keeps the drain to one
    # HWDGE issue
    out_d = nc.declare_dram_parameter("out", [128, 4], F32, isOutput=True)

    with tile.TileContext(nc) as tc:
        with (
            tc.tile_pool(name="const", bufs=1) as const,
            tc.tile_pool(name="decp", bufs=2) as decp,
            tc.tile_pool(name="work", bufs=3) as work,
            tc.tile_pool(name="psum", bufs=1, space="PSUM") as psum,
        ):
            wpack = const.tile([128, 385], BF16, tag="wpack", name="wpack")
            bpack = const.tile([128, 4], F32, tag="bpack", name="bpack")
            nc.sync.dma_start(out=wpack[:], in_=wpack_d[:])
            wfi = wpack[:, 0:128]
            wif = wpack[:, 128:256]
            wg = wpack[:, 256:320]
            wo = wpack[:, 320:384]
            # fc2 replicated at partitions 64:128 so its base partition
            # matches the h rows read directly out of xh (rows 64:128)
            fc2 = wpack[HID:128, 384:385]
            bfi = bpack[:, 0:1]
            bif = bpack[:, 1:2]
            bg = bpack[:, 2:3]
            bo = bpack[:, 3:4]

            # persistent state: ping/pong xh per half, packed cell state.
            # step 0 runs h=c=0 specialized (K=64 x-only matmuls, c2 = i*g),
            # so no state memsets and no dec[0] transfer are needed.
            xh = [
                [
                    const.tile([128, HALF], BF16, tag=f"xh{q}{p}", name=f"xh{q}{p}")
                    for p in range(2)
                ]
                for q in range(2)
            ]
            c2 = const.tile([128, HALF], BF16, tag="c2", name="c2")
            # final-step h, both halves dense (rows 0:64 = half0, 64:128 =
            # half1); only the FC reads it
            hfin = const.tile([128, HALF], BF16, tag="hfin", name="hfin")

            # PE p-state warmup: one garbage matmul on a zeroed scratch tile
            # starts the tensor engine's clock ramp ~3us before the first
            # real matmul lands, so step 0 runs at full speed (the scratch
            # init rides the otherwise-idle ACT engine)
            warm = const.tile([128, 512], BF16, tag="warm", name="warm")
            nc.scalar.memzero(warm[:])
            pwarm = psum.tile([128, 512], F32, tag="pA0", name="pwarm")
            nc.tensor.matmul(
                pwarm[0:64, :], warm[:, 0:64], warm[:], start=True, stop=True
            )

            NCH = 2  # free-dim chunks per half (each with its own psum slots)
            CW = HALF // NCH
            NBC = CW // 512
            for t in range(STEP):
                par = t % 2
                x0, x1 = xh[0][par], xh[1][par]
                n0, n1 = xh[0][1 - par], xh[1][1 - par]
                if t == 0:
                    # chunk-granular priming so the first matmuls/acts start
                    # as soon as the first quarter of step-0 data lands;
                    # biases ship right after the first chunk's x (they are
                    # only needed once the first sigmoid runs)
                    for ch in range(NCH):
                        chs = bass.ds(ch * CW, CW)
                        nc.sync.dma_start(
                            out=x0[0:DIM, chs], in_=x_d[t, :, bass.ds(ch * CW, CW)]
                        )
                        nc.sync.dma_start(
                            out=x1[0:DIM, chs],
                            in_=x_d[t, :, bass.ds(HALF + ch * CW, CW)],
                        )
                        if ch == 0:
                            nc.sync.dma_start(out=bpack[:], in_=bpack_d[:])
                else:
                    decb = decp.tile([128, HALF], BF16, tag="decb", name="decb")
                    nc.sync.dma_start(out=decb[:], in_=dec_d[t])
                    nc.sync.dma_start(out=x0[0:DIM, :], in_=x_d[t, :, bass.ts(0, HALF)])
                    nc.sync.dma_start(out=x1[0:DIM, :], in_=x_d[t, :, bass.ts(1, HALF)])

                # step 0: h rows of xh are uninitialized; contract over x only
                KC = DIM if t == 0 else 128
                wfiK, wifK, wgK, woK = wfi[0:KC], wif[0:KC], wg[0:KC], wo[0:KC]

                for ch in range(NCH):
                    cs = bass.ds(ch * CW, CW)
                    pif0 = psum.tile([128, CW], F32, tag=f"pA{ch}", name="pif0")
                    for j in range(NBC):
                        js = bass.ds(ch * CW + j * 512, 512)
                        ps = bass.ts(j, 512)
                        nc.tensor.matmul(
                            pif0[:, ps], wfiK[:], x0[0:KC, js], start=True, stop=True
                        )
                    tg2 = psum.tile([128, CW], F32, tag=f"pB{ch}", name="tg2")
                    for j in range(NBC):
                        js = bass.ds(ch * CW + j * 512, 512)
                        ps = bass.ts(j, 512)
                        nc.tensor.matmul(
                            tg2[0:HID, ps], wgK[:], x1[0:KC, js], start=True, stop=True
                        )
                        nc.tensor.matmul(
                            tg2[HID:128, ps], wgK[:], x0[0:KC, js], start=True, stop=True
                        )
                    sif0 = work.tile([128, HALF], BF16, tag="sif0", name="sif0")
                    nc.scalar.activation(
                        sif0[:, cs], pif0[:], AF.Sigmoid, bias=bfi[:]
                    )

                    pif1 = psum.tile([128, CW], F32, tag=f"pA{ch}", name="pif1")
                    for j in range(NBC):
                        js = bass.ds(ch * CW + j * 512, 512)
                        ps = bass.ts(j, 512)
                        nc.tensor.matmul(
                            pif1[:, ps], wifK[:], x1[0:KC, js], start=True, stop=True
                        )
                    tgs = work.tile([128, HALF], BF16, tag="tgs", name="tgs")
                    nc.scalar.activation(tgs[:, cs], tg2[:], AF.Tanh, bias=bg[:])

                    poo = psum.tile([128, CW], F32, tag=f"pB{ch}", name="poo")
                    for j in range(NBC):
                        js = bass.ds(ch * CW + j * 512, 512)
                        ps = bass.ts(j, 512)
                        nc.tensor.matmul(
                            poo[0:HID, ps], woK[:], x0[0:KC, js], start=True, stop=True
                        )
                        nc.tensor.matmul(
                            poo[HID:128, ps], woK[:], x1[0:KC, js], start=True, stop=True
                        )
                    sif1 = work.tile([128, HALF], BF16, tag="sif1", name="sif1")
                    nc.scalar.activation(
                        sif1[:, cs], pif1[:], AF.Sigmoid, bias=bif[:]
                    )
                    so2 = work.tile([128, HALF], BF16, tag="so2", name="so2")
                    nc.scalar.activation(so2[:, cs], poo[:], AF.Sigmoid, bias=bo[:])

                    if t == 0:
                        # c0 = 0: cell state is just i*g, written straight
                        # into c2 (no decay/forget path this step)
                        nc.vector.tensor_mul(
                            c2[0:HID, cs], sif0[HID:128, cs], tgs[HID:128, cs]
                        )
                        nc.vector.tensor_mul(
                            c2[HID:128, cs], sif1[0:HID, cs], tgs[0:HID, cs]
                        )
                    else:
                        # DVE cell update (bases matched per op)
                        dc2 = work.tile([128, HALF], BF16, tag="dc2", name="dc2")
                        nc.gpsimd.tensor_mul(dc2[:, cs], c2[:, cs], decb[:, cs])
                        igT = work.tile([128, HALF], BF16, tag="igT", name="igT")
                        fdT = work.tile([128, HALF], BF16, tag="fdT", name="fdT")
                        # at the very last chunk of the last step the cell
                        # update runs 512-col-granular so tanh/hfin can chase
                        # the first half down the drain; everywhere else one
                        # dense pass per op. (i at rows 64:128 of sif0 /
                        # 0:64 of sif1 per the permuted gate packing.)
                        halves = (
                            [bass.ds(ch * CW, 512), bass.ds(ch * CW + 512, 512)]
                            if (t == STEP - 1 and ch == 1)
                            else [cs]
                        )
                        for hsl in halves:
                            nc.vector.tensor_mul(
                                igT[0:HID, hsl], sif0[HID:128, hsl], tgs[HID:128, hsl]
                            )
                            nc.vector.tensor_mul(
                                igT[HID:128, hsl], sif1[0:HID, hsl], tgs[0:HID, hsl]
                            )
                            nc.vector.tensor_mul(
                                fdT[0:HID, hsl], sif0[0:HID, hsl], dc2[0:HID, hsl]
                            )
                            nc.vector.tensor_mul(
                                fdT[HID:128, hsl], sif1[HID:128, hsl], dc2[HID:128, hsl]
                            )
                            nc.vector.tensor_add(c2[:, hsl], igT[:, hsl], fdT[:, hsl])
                    tch = work.tile([128, HALF], BF16, tag="tch", name="tch")
                    if t == STEP - 1 and ch == 1:
                        for hoff in (0, 512):
                            hs = bass.ds(ch * CW + hoff, 512)
                            nc.scalar.activation(tch[:, hs], c2[:, hs], AF.Tanh)
                    else:
                        nc.scalar.activation(tch[:, cs], c2[:, cs], AF.Tanh)
                    if t == STEP - 1:
                        # no next step: h feeds only the FC, so both halves'
                        # h land straight in the dedicated hfin tile
                        if ch == 1:
                            for hoff in (0, 512):
                                hs = bass.ds(ch * CW + hoff, 512)
                                nc.vector.tensor_mul(
                                    hfin[:, hs], so2[:, hs], tch[:, hs]
                                )
                        else:
                            nc.vector.tensor_mul(
                                hfin[:, cs], so2[:, cs], tch[:, cs]
                            )
                    else:
                        nc.vector.tensor_mul(
                            n0[HID:128, cs], so2[0:HID, cs], tch[0:HID, cs]
                        )
                        nc.vector.tensor_mul(
                            n1[HID:128, cs], so2[HID:128, cs], tch[HID:128, cs]
                        )

            # ---- final: q = 1 - sigmoid(h@w + b), noisy-OR over nodules ----
            # h is read straight out of the final-parity xh tiles (rows 64:
            # 128); FC matmuls/sigmoids are emitted chunk0-cols first so they
            # overlap step-31 chunk1 compute, with 2 psum tags ping-ponged.
            fpar = STEP % 2
            nb2 = const.tile([128, 1], F32, tag="nb2", name="nb2")
            nc.vector.memset(nb2[:], -fc2_b)
            # transposed FC: per block of 128 b-samples, 8 matmuls use the
            # nodule-strided h columns as the STATIONARY operand and fc2 as
            # the 1-wide moving operand, landing z[b-sample, nodule] as
            # [128, 8] in PSUM. PE cost tracks output width (1), so the
            # matmuls are ~free; the sigmoid collapses to [128,8] (192ns vs
            # 1038) and the noisy-OR tree runs at free size 4/2/1.
            t3a = const.tile([128, 4], F32, tag="t3a", name="t3a")
            t3av = t3a[0:128].rearrange("p (q c) -> p q c", c=2)
            # ch-major, with both halves of a chunk-pair fused into one
            # [128,16] sigmoid + one 3-op tree (both gate on the same hfin
            # chunk, so fusing removes a sigmoid slot and three semaphore
            # hops from the drain chain)
            for ch in range(NCH):
                cs = bass.ds(ch * CW, CW)
                pz = psum.tile(
                    [128, 2 * NNOD], F32, tag=("pA0" if ch == 0 else "pB0"),
                    name="pz",
                )
                for q in range(2):
                    fc2q = wpack[q * HID : (q + 1) * HID, 384:385]
                    hv = hfin[q * HID : (q + 1) * HID, cs].rearrange(
                        "p (b n) -> p n b", n=NNOD
                    )
                    for n in range(NNOD):
                        nc.tensor.matmul(
                            pz[:, q * NNOD + n : q * NNOD + n + 1],
                            hv[:, n, :],
                            fc2q[:],
                            start=True,
                            stop=True,
                        )
                qb = work.tile([128, 2 * NNOD], F32, tag="qb", name="qb")
                nc.scalar.activation(
                    qb[:], pz[:], AF.Sigmoid, scale=-1.0, bias=nb2[:]
                )
                q3 = qb[0:128].rearrange("p (b n) -> p b n", n=NNOD)
                t1b = work.tile([128, 8], F32, tag="t1b", name="t1b")
                t13 = t1b[0:128].rearrange("p (b n) -> p b n", n=4)
                nc.vector.tensor_mul(t13[:, :, :], q3[:, :, 0:4], q3[:, :, 4:8])
                t2b = work.tile([128, 4], F32, tag="t2b", name="t2b")
                t23 = t2b[0:128].rearrange("p (b n) -> p b n", n=2)
                nc.vector.tensor_mul(t23[:, :, :], t13[:, :, 0:2], t13[:, :, 2:4])
                nc.vector.tensor_mul(
                    t3av[:, :, ch], t23[:, :, 0:1].rearrange("p b n -> p (b n)"),
                    t23[:, :, 1:2].rearrange("p b n -> p (b n)"),
                )
            # the final affine 1 - k_base*t3 is applied host-side
            nc.sync.dma_start(out=out_d[:], in_=t3a[:])

    _split_multiwaits(nc)
    return nc


def kernel(input, time_dis, w_ih, w_hh, b_ih, b_hh, fc2_w, fc2_b, baseline):
    input = np.asarray(input, dtype=np.float32)
    time_dis = np.asarray(time_dis, dtype=np.float32)
    w_ih = np.asarray(w_ih, dtype=np.float32)
    w_hh = np.asarray(w_hh, dtype=np.float32)
    b_ih = np.asarray(b_ih, dtype=np.float32)
    b_hh = np.asarray(b_hh, dtype=np.float32)
    fc2_w = np.asarray(fc2_w, dtype=np.float32)
    fc2_b = np.asarray(fc2_b, dtype=np.float32)
    baseline = np.asarray(baseline, dtype=np.float32)

    bf = ml_dtypes.bfloat16
    bper = BSIZE // NCORES  # 512

    # gates^T = W^T.T @ [x;h], W = [w_ih | w_hh]  [256, 128]
    W = np.concatenate([w_ih, w_hh], axis=1)  # [256, 128]
    lhsT = np.ascontiguousarray(W.T)  # [128, 256] cols: i(0:64) f g o
    li, lf = lhsT[:, 0:64], lhsT[:, 64:128]
    lg, lo = lhsT[:, 128:192], lhsT[:, 192:256]
    wfi = np.concatenate([lf, li], axis=1)
    wif = np.concatenate([li, lf], axis=1)
    fc2col = np.zeros((128, 1), dtype=np.float32)
    fc2col[0:HID, 0] = fc2_w.reshape(HID)
    fc2col[HID:128, 0] = fc2_w.reshape(HID)
    wpack = np.ascontiguousarray(
        np.concatenate([wfi, wif, lg, lo, fc2col], axis=1)
    ).astype(bf)  # [128, 385]
    bias = (b_ih + b_hh).astype(np.float32)
    bi, bfg = bias[0:64], bias[64:128]
    bgg, bog = bias[128:192], bias[192:256]
    bpack = np.ascontiguousarray(
        np.stack(
            [
                np.concatenate([bfg, bi]),
                np.concatenate([bi, bfg]),
                np.concatenate([bgg, bgg]),
                np.concatenate([bog, bog]),
            ],
            axis=1,
        )
    )  # [128, 4] f32
    k_base = float(1.0 - 1.0 / (1.0 + math.exp(-float(baseline[0]))))

    nc = _build(float(fc2_b[0]), k_base)

    in_maps = []
    for k in range(NCORES):
        bs = slice(k * bper, (k + 1) * bper)
        xs = input[:, bs].reshape(STEP, BL, DIM)
        xs = np.ascontiguousarray(xs.transpose(0, 2, 1)).astype(bf)  # [S,64,BL]
        td = time_dis[bs]  # [512, 32]
        td_bn = np.repeat(td.T, NNOD, axis=1)  # [32, 4096] sample-major
        td_used = np.concatenate([td_bn[:1], td_bn[:-1]], axis=0)
        dec = (1.0 / np.log(math.e + td_used)).astype(bf)  # [32, BL]
        # dec2[t, 0:64, j] = dec[t, j] (half0) ; dec2[t, 64:128, j] = dec[t, HALF+j]
        dec2 = np.empty((STEP, 128, HALF), dtype=bf)
        dec2[:, 0:HID, :] = dec[:, None, 0:HALF]
        dec2[:, HID:128, :] = dec[:, None, HALF:BL]
        in_maps.append(
            {
                "x": xs,
                "dec": dec2,
                "wpack": wpack,
                "bpack": bpack,
            }
        )

    res = None
    last_err = None
    for _attempt in range(3):
        try:
            res = run_bass_kernel_spmd(nc, in_maps, list(range(NCORES)))
            break
        except Exception as e:  # transient NRT device errors recover on retry
            last_err = e
    if res is None:
        raise last_err
    global LAST_RESULT
    LAST_RESULT = res
    out = np.concatenate(
        [np.asarray(res.results[k]["out"]).T.reshape(bper) for k in range(NCORES)]
    )
    # device ships t3 = prod_n q; the noisy-OR leak affine runs here
    return (1.0 - k_base * out).astype(np.float32)



# revision 45
# speedup vs baseline: 1.0020x; 1.0004x over previous
"""Trainium2 Bass kernel for nn_DisRNNCellNet (time-decayed LSTM + noisy-OR).

Data-parallel over 8 NeuronCores: bsize 4096 -> 512/core (4096 flat samples
per core, incl. the 8 nodules). Per core a 32-step LSTM (hid=64) runs with
features on SBUF partitions and samples on the free dim, batch split in two
halves of 2048 that share 128-partition-dense ACT/DVE ops:

  pif_h0 [128,2048] = (f,i) gate preacts of half0; pif_h1 = (i,f) of half1
  tg2    [128,2048] = g preacts: rows 0:64 half1, 64:128 half0 (M=64 MMs)
  poo    [128,2048] = o preacts: rows 0:64 half0, 64:128 half1
  c2     [128,2048] = cell state: rows 0:64 half0, 64:128 half1

  ACT (all dense):  sig(pif0) sig(pif1) tanh(tg2) sig(poo) tanh(c2)
  DVE: dc2=c2*dec2 | ig,fdc per half (bases matched) | add | h per half

The gate permutations exist so every 2-input DVE op sees equal input base
partitions (walrus checkSBSameStartPartition). Decay 1/log(e+dt) is host-
precomputed, host-replicated over 64 partitions.

The steady-state loop is ACT-bound (10 dense [128,1024] sigmoid/tanh ops
per step, ~97% ACT occupancy); op granularity is pinned by PSUM (4x 4KB
preact tags) and by the 2-chunk stagger that hides the per-chunk
DVE->tanh(c)->h recurrence latency. Wall-clock trims beyond that come from
the edges: all constants ship in one packed DMA (HWDGE issue is ~625ns
each), step 0 is specialized for h=c=0 (K=64 x-only matmuls, c2=i*g
directly, no state memsets, no dec[0] transfer), a warmup matmul starts the
PE clock ramp early, and the final step writes both halves' h densely into
one hfin tile (no next-step xh split needed). The FC runs TRANSPOSED:
nodule-strided h columns are the stationary operand and fc2 the 1-wide
moving operand, so z lands as [128 b-samples, 8 nodules] — the matmuls cost
~nothing (PE time tracks output width), the sigmoid is [128,8] instead of
[1,1024], the noisy-OR tree runs at free size 4/2/1, and one [128,4] DMA
ships all products (host transposes and applies the 1-k*t3 leak affine).
"""

import math

import ml_dtypes
import numpy as np

import concourse.bass as bass
import concourse.mybir as mybir
import concourse.tile as tile
from concourse.bass_utils import run_bass_kernel_spmd

BF16 = mybir.dt.bfloat16
F32 = mybir.dt.float32
AF = mybir.ActivationFunctionType

STEP, BSIZE, NNOD, DIM, HID = 32, 4096, 8, 64, 64
NCORES = 8
BL = (BSIZE // NCORES) * NNOD  # 4096 flat samples per core
HALF = BL // 2  # 2048
NB = HALF // 512  # 512-wide matmul chunks per half

LAST_RESULT = None


def _split_multiwaits(nc, max_waits=1):
    """walrus in this env rejects >1 sem wait per instruction ("Too many
    sync wait commands"); split extras onto single-wait NoOps."""
    for bb in nc.main_func.blocks:
        out = []
        for ins in bb.instructions:
            si = ins.sync_info
            if si is not None and len(si.on_wait) > max_waits:
                waits = list(si.on_wait)
                for j, w in enumerate(waits[:-max_waits]):
                    out.append(
                        mybir.InstNoOp(
                            name=f"{ins.name}-wsplit{j}",
                            engine=ins.engine,
                            ins=[],
                            outs=[],
                            sync_info=mybir.SyncInfo(on_wait=[w], on_update=[]),
                        )
                    )
                ins.sync_info = mybir.SyncInfo(
                    on_wait=waits[-max_waits:], on_update=list(si.on_update)
                )
            out.append(ins)
        bb.instructions = out


def _build(fc2_b: float, k_base: float):
    nc = bass.Bass(target_bir_lowering=False)
    x_d = nc.declare_dram_parameter("x", [STEP, DIM, BL], BF16, isOutput=False)
    dec_d = nc.declare_dram_parameter("dec", [STEP, 128, HALF], BF16, isOutput=False)
    # all bf16 weights packed in one buffer: wfi|wif|wg|wo|fc2(padded)
    wpack_d = nc.declare_dram_parameter("wpack", [128, 385], BF16, isOutput=False)
    # all f32 biases packed: bfi|bif|bg|bo columns
    bpack_d = nc.declare_dram_parameter("bpack", [128, 4], F32, isOutput=False)
    # noisy-OR products, [b-sample-within-block, block]: host transposes
    # to get core b-sample order; a single tile/DMA keeps the drain to one
    # HWDGE issue
    out_d = nc.declare_dram_parameter("out", [128, 4], F32, isOutput=True)

    with tile.TileContext(nc) as tc:
        with (
            tc.tile_pool(name="const", bufs=1) as const,
            tc.tile_pool(name="decp", bufs=2) as decp,
            tc.tile_pool(name="work", bufs=3) as work,
            tc.tile_pool(name="psum", bufs=1, space="PSUM") as psum,
        ):
            wpack = const.tile([128, 385], BF16, tag="wpack", name="wpack")
            bpack = const.tile([128, 4], F32, tag="bpack", name="bpack")
            nc.sync.dma_start(out=wpack[:], in_=wpack_d[:])
            wfi = wpack[:, 0:128]
            wif = wpack[:, 128:256]
            wg = wpack[:, 256:320]
            wo = wpack[:, 320:384]
            # fc2 replicated at partitions 64:128 so its base partition
            # matches the h rows read directly out of xh (rows 64:128)
            fc2 = wpack[HID:128, 384:385]
            bfi = bpack[:, 0:1]
            bif = bpack[:, 1:2]
            bg = bpack[:, 2:3]
            bo = bpack[:, 3:4]

            # persistent state: ping/pong xh per half, packed cell state.
            # step 0 runs h=c=0 specialized (K=64 x-only matmuls, c2 = i*g),
            # so no state memsets and no dec[0] transfer are needed.
            xh = [
                [
                    const.tile([128, HALF], BF16, tag=f"xh{q}{p}", name=f"xh{q}{p}")
                    for p in range(2)
                ]
                for q in range(2)
            ]
            c2 = const.tile([128, HALF], BF16, tag="c2", name="c2")
            # final-step h, both halves dense (rows 0:64 = half0, 64:128 =
            # half1); only the FC reads it
            hfin = const.tile([128, HALF], BF16, tag="hfin", name="hfin")

            # PE p-state warmup: one garbage matmul on a zeroed scratch tile
            # starts the tensor engine's clock ramp ~3us before the first
            # real matmul lands, so step 0 runs at full speed (the scratch
            # init rides the otherwise-idle ACT engine)
            warm = const.tile([128, 512], BF16, tag="warm", name="warm")
            nc.scalar.memzero(warm[:])
            pwarm = psum.tile([128, 512], F32, tag="pA0", name="pwarm")
            nc.tensor.matmul(
                pwarm[0:64, :], warm[:, 0:64], warm[:], start=True, stop=True
            )

            NCH = 2  # free-dim chunks per half (each with its own psum slots)
            CW = HALF // NCH
            NBC = CW // 512
            for t in range(STEP):
                par = t % 2
                x0, x1 = xh[0][par], xh[1][par]
                n0, n1 = xh[0][1 - par], xh[1][1 - par]
                if t == 0:
                    # chunk-granular priming so the first matmuls/acts start
                    # as soon as the first quarter of step-0 data lands;
                    # biases ship right after the first chunk's x (they are
                    # only needed once the first sigmoid runs)
                    for ch in range(NCH):
                        chs = bass.ds(ch * CW, CW)
                        nc.sync.dma_start(
                            out=x0[0:DIM, chs], in_=x_d[t, :, bass.ds(ch * CW, CW)]
                        )
                        nc.sync.dma_start(
                            out=x1[0:DIM, chs],
                            in_=x_d[t, :, bass.ds(HALF + ch * CW, CW)],
                        )
                        if ch == 0:
                            nc.sync.dma_start(out=bpack[:], in_=bpack_d[:])
                else:
                    decb = decp.tile([128, HALF], BF16, tag="decb", name="decb")
                    nc.sync.dma_start(out=decb[:], in_=dec_d[t])
                    nc.sync.dma_start(out=x0[0:DIM, :], in_=x_d[t, :, bass.ts(0, HALF)])
                    nc.sync.dma_start(out=x1[0:DIM, :], in_=x_d[t, :, bass.ts(1, HALF)])

                # step 0: h rows of xh are uninitialized; contract over x only
                KC = DIM if t == 0 else 128
                wfiK, wifK, wgK, woK = wfi[0:KC], wif[0:KC], wg[0:KC], wo[0:KC]

                for ch in range(NCH):
                    cs = bass.ds(ch * CW, CW)
                    pif0 = psum.tile([128, CW], F32, tag=f"pA{ch}", name="pif0")
                    for j in range(NBC):
                        js = bass.ds(ch * CW + j * 512, 512)
                        ps = bass.ts(j, 512)
                        nc.tensor.matmul(
                            pif0[:, ps], wfiK[:], x0[0:KC, js], start=True, stop=True
                        )
                    tg2 = psum.tile([128, CW], F32, tag=f"pB{ch}", name="tg2")
                    for j in range(NBC):
                        js = bass.ds(ch * CW + j * 512, 512)
                        ps = bass.ts(j, 512)
                        nc.tensor.matmul(
                            tg2[0:HID, ps], wgK[:], x1[0:KC, js], start=True, stop=True
                        )
                        nc.tensor.matmul(
                            tg2[HID:128, ps], wgK[:], x0[0:KC, js], start=True, stop=True
                        )
                    sif0 = work.tile([128, HALF], BF16, tag="sif0", name="sif0")
                    nc.scalar.activation(
                        sif0[:, cs], pif0[:], AF.Sigmoid, bias=bfi[:]
                    )

                    pif1 = psum.tile([128, CW], F32, tag=f"pA{ch}", name="pif1")
                    for j in range(NBC):
                        js = bass.ds(ch * CW + j * 512, 512)
                        ps = bass.ts(j, 512)
                        nc.tensor.matmul(
                            pif1[:, ps], wifK[:], x1[0:KC, js], start=True, stop=True
                        )
                    tgs = work.tile([128, HALF], BF16, tag="tgs", name="tgs")
                    nc.scalar.activation(tgs[:, cs], tg2[:], AF.Tanh, bias=bg[:])

                    poo = psum.tile([128, CW], F32, tag=f"pB{ch}", name="poo")
                    for j in range(NBC):
                        js = bass.ds(ch * CW + j * 512, 512)
                        ps = bass.ts(j, 512)
                        nc.tensor.matmul(
                            poo[0:HID, ps], woK[:], x0[0:KC, js], start=True, stop=True
                        )
                        nc.tensor.matmul(
                            poo[HID:128, ps], woK[:], x1[0:KC, js], start=True, stop=True
                        )
                    sif1 = work.tile([128, HALF], BF16, tag="sif1", name="sif1")
                    so2 = work.tile([128, HALF], BF16, tag="so2", name="so2")
                    if t == STEP - 1 and ch == 1:
                        # drain: halve the last two stream sigmoids so the
                        # final cell-update chain starts half an op earlier
                        for hoff in (0, 512):
                            hs = bass.ds(ch * CW + hoff, 512)
                            ps = bass.ds(hoff, 512)
                            nc.scalar.activation(
                                sif1[:, hs], pif1[:, ps], AF.Sigmoid, bias=bif[:]
                            )
                        for hoff in (0, 512):
                            hs = bass.ds(ch * CW + hoff, 512)
                            ps = bass.ds(hoff, 512)
                            nc.scalar.activation(
                                so2[:, hs], poo[:, ps], AF.Sigmoid, bias=bo[:]
                            )
                    else:
                        nc.scalar.activation(
                            sif1[:, cs], pif1[:], AF.Sigmoid, bias=bif[:]
                        )
                        nc.scalar.activation(so2[:, cs], poo[:], AF.Sigmoid, bias=bo[:])

                    if t == 0:
                        # c0 = 0: cell state is just i*g, written straight
                        # into c2 (no decay/forget path this step)
                        nc.vector.tensor_mul(
                            c2[0:HID, cs], sif0[HID:128, cs], tgs[HID:128, cs]
                        )
                        nc.vector.tensor_mul(
                            c2[HID:128, cs], sif1[0:HID, cs], tgs[0:HID, cs]
                        )
                    else:
                        # DVE cell update (bases matched per op)
                        dc2 = work.tile([128, HALF], BF16, tag="dc2", name="dc2")
                        nc.gpsimd.tensor_mul(dc2[:, cs], c2[:, cs], decb[:, cs])
                        igT = work.tile([128, HALF], BF16, tag="igT", name="igT")
                        fdT = work.tile([128, HALF], BF16, tag="fdT", name="fdT")
                        # at the very last chunk of the last step the cell
                        # update runs 512-col-granular so tanh/hfin can chase
                        # the first half down the drain; everywhere else one
                        # dense pass per op. (i at rows 64:128 of sif0 /
                        # 0:64 of sif1 per the permuted gate packing.)
                        halves = (
                            [bass.ds(ch * CW, 512), bass.ds(ch * CW + 512, 512)]
                            if (t == STEP - 1 and ch == 1)
                            else [cs]
                        )
                        for hsl in halves:
                            nc.vector.tensor_mul(
                                igT[0:HID, hsl], sif0[HID:128, hsl], tgs[HID:128, hsl]
                            )
                            nc.vector.tensor_mul(
                                igT[HID:128, hsl], sif1[0:HID, hsl], tgs[0:HID, hsl]
                            )
                            nc.vector.tensor_mul(
                                fdT[0:HID, hsl], sif0[0:HID, hsl], dc2[0:HID, hsl]
                            )
                            nc.vector.tensor_mul(
                                fdT[HID:128, hsl], sif1[HID:128, hsl], dc2[HID:128, hsl]
                            )
                            nc.vector.tensor_add(c2[:, hsl], igT[:, hsl], fdT[:, hsl])
                    tch = work.tile([128, HALF], BF16, tag="tch", name="tch")
                    if t == STEP - 1 and ch == 1:
                        for hoff in (0, 512):
                            hs = bass.ds(ch * CW + hoff, 512)
                            nc.scalar.activation(tch[:, hs], c2[:, hs], AF.Tanh)
                    else:
                        nc.scalar.activation(tch[:, cs], c2[:, cs], AF.Tanh)
                    if t == STEP - 1:
                        # no next step: h feeds only the FC, so both halves'
                        # h land straight in the dedicated hfin tile
                        if ch == 1:
                            for hoff in (0, 512):
                                hs = bass.ds(ch * CW + hoff, 512)
                                nc.vector.tensor_mul(
                                    hfin[:, hs], so2[:, hs], tch[:, hs]
                                )
                        else:
                            nc.vector.tensor_mul(
                                hfin[:, cs], so2[:, cs], tch[:, cs]
                            )
                    else:
                        nc.vector.tensor_mul(
                            n0[HID:128, cs], so2[0:HID, cs], tch[0:HID, cs]
                        )
                        nc.vector.tensor_mul(
                            n1[HID:128, cs], so2[HID:128, cs], tch[HID:128, cs]
                        )

            # ---- final: q = 1 - sigmoid(h@w + b), noisy-OR over nodules ----
            # h is read straight out of the final-parity xh tiles (rows 64:
            # 128); FC matmuls/sigmoids are emitted chunk0-cols first so they
            # overlap step-31 chunk1 compute, with 2 psum tags ping-ponged.
            fpar = STEP % 2
            nb2 = const.tile([128, 1], F32, tag="nb2", name="nb2")
            nc.vector.memset(nb2[:], -fc2_b)
            # transposed FC: per block of 128 b-samples, 8 matmuls use the
            # nodule-strided h columns as the STATIONARY operand and fc2 as
            # the 1-wide moving operand, landing z[b-sample, nodule] as
            # [128, 8] in PSUM. PE cost tracks output width (1), so the
            # matmuls are ~free; the sigmoid collapses to [128,8] (192ns vs
            # 1038) and the noisy-OR tree runs at free size 4/2/1.
            t3a = const.tile([128, 4], F32, tag="t3a", name="t3a")
            t3av = t3a[0:128].rearrange("p (q c) -> p q c", c=2)
            # ch-major, with both halves of a chunk-pair fused into one
            # [128,16] sigmoid + one 3-op tree (both gate on the same hfin
            # chunk, so fusing removes a sigmoid slot and three semaphore
            # hops from the drain chain)
            for ch in range(NCH):
                cs = bass.ds(ch * CW, CW)
                pz = psum.tile(
                    [128, 2 * NNOD], F32, tag=("pA0" if ch == 0 else "pB0"),
                    name="pz",
                )
                for q in range(2):
                    fc2q = wpack[q * HID : (q + 1) * HID, 384:385]
                    hv = hfin[q * HID : (q + 1) * HID, cs].rearrange(
                        "p (b n) -> p n b", n=NNOD
                    )
                    for n in range(NNOD):
                        nc.tensor.matmul(
                            pz[:, q * NNOD + n : q * NNOD + n + 1],
                            hv[:, n, :],
                            fc2q[:],
                            start=True,
                            stop=True,
                        )
                qb = work.tile([128, 2 * NNOD], F32, tag="qb", name="qb")
                nc.scalar.activation(
                    qb[:], pz[:], AF.Sigmoid, scale=-1.0, bias=nb2[:]
                )
                q3 = qb[0:128].rearrange("p (b n) -> p b n", n=NNOD)
                t1b = work.tile([128, 8], F32, tag="t1b", name="t1b")
                t13 = t1b[0:128].rearrange("p (b n) -> p b n", n=4)
                nc.vector.tensor_mul(t13[:, :, :], q3[:, :, 0:4], q3[:, :, 4:8])
                t2b = work.tile([128, 4], F32, tag="t2b", name="t2b")
                t23 = t2b[0:128].rearrange("p (b n) -> p b n", n=2)
                nc.vector.tensor_mul(t23[:, :, :], t13[:, :, 0:2], t13[:, :, 2:4])
                nc.vector.tensor_mul(
                    t3av[:, :, ch], t23[:, :, 0:1].rearrange("p b n -> p (b n)"),
                    t23[:, :, 1:2].rearrange("p b n -> p (b n)"),
                )
            # the final affine 1 - k_base*t3 is applied host-side
            nc.sync.dma_start(out=out_d[:], in_=t3a[:])

    _split_multiwaits(nc)
    return nc


def kernel(input, time_dis, w_ih, w_hh, b_ih, b_hh, fc2_w, fc2_b, baseline):
    input = np.asarray(input, dtype=np.float32)
    time_dis = np.asarray(time_dis, dtype=np.float32)
    w_ih = np.asarray(w_ih, dtype=np.float32)
    w_hh = np.asarray(w_hh, dtype=np.float32)
    b_ih = np.asarray(b_ih, dtype=np.float32)
    b_hh = np.asarray(b_hh, dtype=np.float32)
    fc2_w = np.asarray(fc2_w, dtype=np.float32)
    fc2_b = np.asarray(fc2_b, dtype=np.float32)
    baseline = np.asarray(baseline, dtype=np.float32)

    bf = ml_dtypes.bfloat16
    bper = BSIZE // NCORES  # 512

    # gates^T = W^T.T @ [x;h], W = [w_ih | w_hh]  [256, 128]
    W = np.concatenate([w_ih, w_hh], axis=1)  # [256, 128]
    lhsT = np.ascontiguousarray(W.T)  # [128, 256] cols: i(0:64) f g o
    li, lf = lhsT[:, 0:64], lhsT[:, 64:128]
    lg, lo = lhsT[:, 128:192], lhsT[:, 192:256]
    wfi = np.concatenate([lf, li], axis=1)
    wif = np.concatenate([li, lf], axis=1)
    fc2col = np.zeros((128, 1), dtype=np.float32)
    fc2col[0:HID, 0] = fc2_w.reshape(HID)
    fc2col[HID:128, 0] = fc2_w.reshape(HID)
    wpack = np.ascontiguousarray(
        np.concatenate([wfi, wif, lg, lo, fc2col], axis=1)
    ).astype(bf)  # [128, 385]
    bias = (b_ih + b_hh).astype(np.float32)
    bi, bfg = bias[0:64], bias[64:128]
    bgg, bog = bias[128:192], bias[192:256]
    bpack = np.ascontiguousarray(
        np.stack(
            [
                np.concatenate([bfg, bi]),
                np.concatenate([bi, bfg]),
                np.concatenate([bgg, bgg]),
                np.concatenate([bog, bog]),
            ],
            axis=1,
        )
    )  # [128, 4] f32
    k_base = float(1.0 - 1.0 / (1.0 + math.exp(-float(baseline[0]))))

    nc = _build(float(fc2_b[0]), k_base)

    in_maps = []
    for k in range(NCORES):
        bs = slice(k * bper, (k + 1) * bper)
        xs = input[:, bs].reshape(STEP, BL, DIM)
        xs = np.ascontiguousarray(xs.transpose(0, 2, 1)).astype(bf)  # [S,64,BL]
        td = time_dis[bs]  # [512, 32]
        td_bn = np.repeat(td.T, NNOD, axis=1)  # [32, 4096] sample-major
        td_used = np.concatenate([td_bn[:1], td_bn[:-1]], axis=0)
        dec = (1.0 / np.log(math.e + td_used)).astype(bf)  # [32, BL]
        # dec2[t, 0:64, j] = dec[t, j] (half0) ; dec2[t, 64:128, j] = dec[t, HALF+j]
        dec2 = np.empty((STEP, 128, HALF), dtype=bf)
        dec2[:, 0:HID, :] = dec[:, None, 0:HALF]
        dec2[:, HID:128, :] = dec[:, None, HALF:BL]
        in_maps.append(
            {
                "x": xs,
                "dec": dec2,
                "wpack": wpack,
                "bpack": bpack,
            }
        )

    res = None
    last_err = None
    for _attempt in range(3):
        try:
            res = run_bass_kernel_spmd(nc, in_maps, list(range(NCORES)))
            break
        except Exception as e:  # transient NRT device errors recover on retry
            last_err = e
    if res is None:
        raise last_err
    global LAST_RESULT
    LAST_RESULT = res
    out = np.concatenate(
        [np.asarray(res.results[k]["out"]).T.reshape(bper) for k in range(NCORES)]
    )
    # device ships t3 = prod_n q; the noisy-OR leak affine runs here
    return (1.0 - k_base * out).astype(np.float32)



# revision 46
# speedup vs baseline: 1.0029x; 1.0008x over previous
"""Trainium2 Bass kernel for nn_DisRNNCellNet (time-decayed LSTM + noisy-OR).

Data-parallel over 8 NeuronCores: bsize 4096 -> 512/core (4096 flat samples
per core, incl. the 8 nodules). Per core a 32-step LSTM (hid=64) runs with
features on SBUF partitions and samples on the free dim, batch split in two
halves of 2048 that share 128-partition-dense ACT/DVE ops:

  pif_h0 [128,2048] = (f,i) gate preacts of half0; pif_h1 = (i,f) of half1
  tg2    [128,2048] = g preacts: rows 0:64 half1, 64:128 half0 (M=64 MMs)
  poo    [128,2048] = o preacts: rows 0:64 half0, 64:128 half1
  c2     [128,2048] = cell state: rows 0:64 half0, 64:128 half1

  ACT (all dense):  sig(pif0) sig(pif1) tanh(tg2) sig(poo) tanh(c2)
  DVE: dc2=c2*dec2 | ig,fdc per half (bases matched) | add | h per half

The gate permutations exist so every 2-input DVE op sees equal input base
partitions (walrus checkSBSameStartPartition). Decay 1/log(e+dt) is host-
precomputed, host-replicated over 64 partitions.

The steady-state loop is ACT-bound (10 dense [128,1024] sigmoid/tanh ops
per step, ~97% ACT occupancy); op granularity is pinned by PSUM (4x 4KB
preact tags) and by the 2-chunk stagger that hides the per-chunk
DVE->tanh(c)->h recurrence latency. Wall-clock trims beyond that come from
the edges: all constants ship in one packed DMA (HWDGE issue is ~625ns
each), step 0 is specialized for h=c=0 (K=64 x-only matmuls, c2=i*g
directly, no state memsets, no dec[0] transfer), a warmup matmul starts the
PE clock ramp early, and the final step writes both halves' h densely into
one hfin tile (no next-step xh split needed). The FC runs TRANSPOSED:
nodule-strided h columns are the stationary operand and fc2 the 1-wide
moving operand, so z lands as [128 b-samples, 8 nodules] — the matmuls cost
~nothing (PE time tracks output width), the sigmoid is [128,8] instead of
[1,1024], the noisy-OR tree runs at free size 4/2/1, and one [128,4] DMA
ships all products (host transposes and applies the 1-k*t3 leak affine).
"""

import math

import ml_dtypes
import numpy as np

import concourse.bass as bass
import concourse.mybir as mybir
import concourse.tile as tile
from concourse.bass_utils import run_bass_kernel_spmd

BF16 = mybir.dt.bfloat16
F32 = mybir.dt.float32
AF = mybir.ActivationFunctionType

STEP, BSIZE, NNOD, DIM, HID = 32, 4096, 8, 64, 64
NCORES = 8
BL = (BSIZE // NCORES) * NNOD  # 4096 flat samples per core
HALF = BL // 2  # 2048
NB = HALF // 512  # 512-wide matmul chunks per half

LAST_RESULT = None


def _split_multiwaits(nc, max_waits=1):
    """walrus in this env rejects >1 sem wait per instruction ("Too many
    sync wait commands"); split extras onto single-wait NoOps."""
    for bb in nc.main_func.blocks:
        out = []
        for ins in bb.instructions:
            si = ins.sync_info
            if si is not None and len(si.on_wait) > max_waits:
                waits = list(si.on_wait)
                for j, w in enumerate(waits[:-max_waits]):
                    out.append(
                        mybir.InstNoOp(
                            name=f"{ins.name}-wsplit{j}",
                            engine=ins.engine,
                            ins=[],
                            outs=[],
                            sync_info=mybir.SyncInfo(on_wait=[w], on_update=[]),
                        )
                    )
                ins.sync_info = mybir.SyncInfo(
                    on_wait=waits[-max_waits:], on_update=list(si.on_update)
                )
            out.append(ins)
        bb.instructions = out


def _build(fc2_b: float, k_base: float):
    nc = bass.Bass(target_bir_lowering=False)
    x_d = nc.declare_dram_parameter("x", [STEP, DIM, BL], BF16, isOutput=False)
    dec_d = nc.declare_dram_parameter("dec", [STEP, 128, HALF], BF16, isOutput=False)
    # all bf16 weights packed in one buffer: wfi|wif|wg|wo|fc2(padded)
    wpack_d = nc.declare_dram_parameter("wpack", [128, 385], BF16, isOutput=False)
    # all f32 biases packed: bfi|bif|bg|bo columns
    bpack_d = nc.declare_dram_parameter("bpack", [128, 4], F32, isOutput=False)
    # noisy-OR products, [b-sample-within-block, block]: host transposes
    # to get core b-sample order; a single tile/DMA keeps the drain to one
    # HWDGE issue
    out_d = nc.declare_dram_parameter("out", [128, 4], F32, isOutput=True)

    with tile.TileContext(nc) as tc:
        with (
            tc.tile_pool(name="const", bufs=1) as const,
            tc.tile_pool(name="decp", bufs=2) as decp,
            tc.tile_pool(name="work", bufs=3) as work,
            tc.tile_pool(name="psum", bufs=1, space="PSUM") as psum,
        ):
            wpack = const.tile([128, 385], BF16, tag="wpack", name="wpack")
            bpack = const.tile([128, 4], F32, tag="bpack", name="bpack")
            nc.sync.dma_start(out=wpack[:], in_=wpack_d[:])
            wfi = wpack[:, 0:128]
            wif = wpack[:, 128:256]
            wg = wpack[:, 256:320]
            wo = wpack[:, 320:384]
            # fc2 replicated at partitions 64:128 so its base partition
            # matches the h rows read directly out of xh (rows 64:128)
            fc2 = wpack[HID:128, 384:385]
            bfi = bpack[:, 0:1]
            bif = bpack[:, 1:2]
            bg = bpack[:, 2:3]
            bo = bpack[:, 3:4]

            # persistent state: ping/pong xh per half, packed cell state.
            # step 0 runs h=c=0 specialized (K=64 x-only matmuls, c2 = i*g),
            # so no state memsets and no dec[0] transfer are needed.
            xh = [
                [
                    const.tile([128, HALF], BF16, tag=f"xh{q}{p}", name=f"xh{q}{p}")
                    for p in range(2)
                ]
                for q in range(2)
            ]
            c2 = const.tile([128, HALF], BF16, tag="c2", name="c2")
            # final-step h, both halves dense (rows 0:64 = half0, 64:128 =
            # half1); only the FC reads it
            hfin = const.tile([128, HALF], BF16, tag="hfin", name="hfin")

            # PE p-state warmup: one garbage matmul on a zeroed scratch tile
            # starts the tensor engine's clock ramp ~3us before the first
            # real matmul lands, so step 0 runs at full speed (the scratch
            # init rides the otherwise-idle ACT engine)
            warm = const.tile([128, 512], BF16, tag="warm", name="warm")
            nc.scalar.memzero(warm[:])
            pwarm = psum.tile([128, 512], F32, tag="pA0", name="pwarm")
            nc.tensor.matmul(
                pwarm[0:64, :], warm[:, 0:64], warm[:], start=True, stop=True
            )

            NCH = 2  # free-dim chunks per half (each with its own psum slots)
            CW = HALF // NCH
            NBC = CW // 512
            for t in range(STEP):
                par = t % 2
                x0, x1 = xh[0][par], xh[1][par]
                n0, n1 = xh[0][1 - par], xh[1][1 - par]
                if t == 0:
                    # chunk-granular priming so the first matmuls/acts start
                    # as soon as the first quarter of step-0 data lands;
                    # biases ship right after the first chunk's x (they are
                    # only needed once the first sigmoid runs)
                    for ch in range(NCH):
                        chs = bass.ds(ch * CW, CW)
                        nc.sync.dma_start(
                            out=x0[0:DIM, chs], in_=x_d[t, :, bass.ds(ch * CW, CW)]
                        )
                        nc.sync.dma_start(
                            out=x1[0:DIM, chs],
                            in_=x_d[t, :, bass.ds(HALF + ch * CW, CW)],
                        )
                        if ch == 0:
                            nc.sync.dma_start(out=bpack[:], in_=bpack_d[:])
                else:
                    decb = decp.tile([128, HALF], BF16, tag="decb", name="decb")
                    nc.sync.dma_start(out=decb[:], in_=dec_d[t])
                    nc.sync.dma_start(out=x0[0:DIM, :], in_=x_d[t, :, bass.ts(0, HALF)])
                    nc.sync.dma_start(out=x1[0:DIM, :], in_=x_d[t, :, bass.ts(1, HALF)])

                # step 0: h rows of xh are uninitialized; contract over x only
                KC = DIM if t == 0 else 128
                wfiK, wifK, wgK, woK = wfi[0:KC], wif[0:KC], wg[0:KC], wo[0:KC]

                for ch in range(NCH):
                    cs = bass.ds(ch * CW, CW)
                    pif0 = psum.tile([128, CW], F32, tag=f"pA{ch}", name="pif0")
                    for j in range(NBC):
                        js = bass.ds(ch * CW + j * 512, 512)
                        ps = bass.ts(j, 512)
                        nc.tensor.matmul(
                            pif0[:, ps], wfiK[:], x0[0:KC, js], start=True, stop=True
                        )
                    tg2 = psum.tile([128, CW], F32, tag=f"pB{ch}", name="tg2")
                    for j in range(NBC):
                        js = bass.ds(ch * CW + j * 512, 512)
                        ps = bass.ts(j, 512)
                        nc.tensor.matmul(
                            tg2[0:HID, ps], wgK[:], x1[0:KC, js], start=True, stop=True
                        )
                        nc.tensor.matmul(
                            tg2[HID:128, ps], wgK[:], x0[0:KC, js], start=True, stop=True
                        )
                    sif0 = work.tile([128, HALF], BF16, tag="sif0", name="sif0")
                    nc.scalar.activation(
                        sif0[:, cs], pif0[:], AF.Sigmoid, bias=bfi[:]
                    )

                    pif1 = psum.tile([128, CW], F32, tag=f"pA{ch}", name="pif1")
                    for j in range(NBC):
                        js = bass.ds(ch * CW + j * 512, 512)
                        ps = bass.ts(j, 512)
                        nc.tensor.matmul(
                            pif1[:, ps], wifK[:], x1[0:KC, js], start=True, stop=True
                        )
                    tgs = work.tile([128, HALF], BF16, tag="tgs", name="tgs")
                    nc.scalar.activation(tgs[:, cs], tg2[:], AF.Tanh, bias=bg[:])

                    poo = psum.tile([128, CW], F32, tag=f"pB{ch}", name="poo")
                    for j in range(NBC):
                        js = bass.ds(ch * CW + j * 512, 512)
                        ps = bass.ts(j, 512)
                        nc.tensor.matmul(
                            poo[0:HID, ps], woK[:], x0[0:KC, js], start=True, stop=True
                        )
                        nc.tensor.matmul(
                            poo[HID:128, ps], woK[:], x1[0:KC, js], start=True, stop=True
                        )
                    sif1 = work.tile([128, HALF], BF16, tag="sif1", name="sif1")
                    so2 = work.tile([128, HALF], BF16, tag="so2", name="so2")
                    if t == STEP - 1 and ch == 1:
                        # drain: halve the last two stream sigmoids so the
                        # final cell-update chain starts half an op earlier
                        for hoff, hw in [(0, 704), (704, 320)]:
                            hs = bass.ds(ch * CW + hoff, hw)
                            ps = bass.ds(hoff, hw)
                            nc.scalar.activation(
                                sif1[:, hs], pif1[:, ps], AF.Sigmoid, bias=bif[:]
                            )
                        for hoff, hw in [(0, 704), (704, 320)]:
                            hs = bass.ds(ch * CW + hoff, hw)
                            ps = bass.ds(hoff, hw)
                            nc.scalar.activation(
                                so2[:, hs], poo[:, ps], AF.Sigmoid, bias=bo[:]
                            )
                    else:
                        nc.scalar.activation(
                            sif1[:, cs], pif1[:], AF.Sigmoid, bias=bif[:]
                        )
                        nc.scalar.activation(so2[:, cs], poo[:], AF.Sigmoid, bias=bo[:])

                    if t == 0:
                        # c0 = 0: cell state is just i*g, written straight
                        # into c2 (no decay/forget path this step)
                        nc.vector.tensor_mul(
                            c2[0:HID, cs], sif0[HID:128, cs], tgs[HID:128, cs]
                        )
                        nc.vector.tensor_mul(
                            c2[HID:128, cs], sif1[0:HID, cs], tgs[0:HID, cs]
                        )
                    else:
                        # DVE cell update (bases matched per op)
                        dc2 = work.tile([128, HALF], BF16, tag="dc2", name="dc2")
                        nc.gpsimd.tensor_mul(dc2[:, cs], c2[:, cs], decb[:, cs])
                        igT = work.tile([128, HALF], BF16, tag="igT", name="igT")
                        fdT = work.tile([128, HALF], BF16, tag="fdT", name="fdT")
                        # at the very last chunk of the last step the cell
                        # update runs 512-col-granular so tanh/hfin can chase
                        # the first half down the drain; everywhere else one
                        # dense pass per op. (i at rows 64:128 of sif0 /
                        # 0:64 of sif1 per the permuted gate packing.)
                        halves = (
                            [bass.ds(ch * CW + o, w) for o, w in [(0, 704), (704, 320)]]
                            if (t == STEP - 1 and ch == 1)
                            else [cs]
                        )
                        for hsl in halves:
                            nc.vector.tensor_mul(
                                igT[0:HID, hsl], sif0[HID:128, hsl], tgs[HID:128, hsl]
                            )
                            nc.vector.tensor_mul(
                                igT[HID:128, hsl], sif1[0:HID, hsl], tgs[0:HID, hsl]
                            )
                            nc.vector.tensor_mul(
                                fdT[0:HID, hsl], sif0[0:HID, hsl], dc2[0:HID, hsl]
                            )
                            nc.vector.tensor_mul(
                                fdT[HID:128, hsl], sif1[HID:128, hsl], dc2[HID:128, hsl]
                            )
                            nc.vector.tensor_add(c2[:, hsl], igT[:, hsl], fdT[:, hsl])
                    tch = work.tile([128, HALF], BF16, tag="tch", name="tch")
                    if t == STEP - 1 and ch == 1:
                        for hoff, hw in [(0, 704), (704, 320)]:
                            hs = bass.ds(ch * CW + hoff, hw)
                            nc.scalar.activation(tch[:, hs], c2[:, hs], AF.Tanh)
                    else:
                        nc.scalar.activation(tch[:, cs], c2[:, cs], AF.Tanh)
                    if t == STEP - 1:
                        # no next step: h feeds only the FC, so both halves'
                        # h land straight in the dedicated hfin tile
                        if ch == 1:
                            for hoff, hw in [(0, 704), (704, 320)]:
                                hs = bass.ds(ch * CW + hoff, hw)
                                nc.vector.tensor_mul(
                                    hfin[:, hs], so2[:, hs], tch[:, hs]
                                )
                        else:
                            nc.vector.tensor_mul(
                                hfin[:, cs], so2[:, cs], tch[:, cs]
                            )
                    else:
                        nc.vector.tensor_mul(
                            n0[HID:128, cs], so2[0:HID, cs], tch[0:HID, cs]
                        )
                        nc.vector.tensor_mul(
                            n1[HID:128, cs], so2[HID:128, cs], tch[HID:128, cs]
                        )

            # ---- final: q = 1 - sigmoid(h@w + b), noisy-OR over nodules ----
            # h is read straight out of the final-parity xh tiles (rows 64:
            # 128); FC matmuls/sigmoids are emitted chunk0-cols first so they
            # overlap step-31 chunk1 compute, with 2 psum tags ping-ponged.
            fpar = STEP % 2
            nb2 = const.tile([128, 1], F32, tag="nb2", name="nb2")
            nc.vector.memset(nb2[:], -fc2_b)
            # transposed FC: per block of 128 b-samples, 8 matmuls use the
            # nodule-strided h columns as the STATIONARY operand and fc2 as
            # the 1-wide moving operand, landing z[b-sample, nodule] as
            # [128, 8] in PSUM. PE cost tracks output width (1), so the
            # matmuls are ~free; the sigmoid collapses to [128,8] (192ns vs
            # 1038) and the noisy-OR tree runs at free size 4/2/1.
            t3a = const.tile([128, 4], F32, tag="t3a", name="t3a")
            t3av = t3a[0:128].rearrange("p (q c) -> p q c", c=2)
            # ch-major, with both halves of a chunk-pair fused into one
            # [128,16] sigmoid + one 3-op tree (both gate on the same hfin
            # chunk, so fusing removes a sigmoid slot and three semaphore
            # hops from the drain chain)
            for ch in range(NCH):
                cs = bass.ds(ch * CW, CW)
                pz = psum.tile(
                    [128, 2 * NNOD], F32, tag=("pA0" if ch == 0 else "pB0"),
                    name="pz",
                )
                for q in range(2):
                    fc2q = wpack[q * HID : (q + 1) * HID, 384:385]
                    hv = hfin[q * HID : (q + 1) * HID, cs].rearrange(
                        "p (b n) -> p n b", n=NNOD
                    )
                    for n in range(NNOD):
                        nc.tensor.matmul(
                            pz[:, q * NNOD + n : q * NNOD + n + 1],
                            hv[:, n, :],
                            fc2q[:],
                            start=True,
                            stop=True,
                        )
                qb = work.tile([128, 2 * NNOD], F32, tag="qb", name="qb")
                nc.scalar.activation(
                    qb[:], pz[:], AF.Sigmoid, scale=-1.0, bias=nb2[:]
                )
                q3 = qb[0:128].rearrange("p (b n) -> p b n", n=NNOD)
                t1b = work.tile([128, 8], F32, tag="t1b", name="t1b")
                t13 = t1b[0:128].rearrange("p (b n) -> p b n", n=4)
                nc.vector.tensor_mul(t13[:, :, :], q3[:, :, 0:4], q3[:, :, 4:8])
                t2b = work.tile([128, 4], F32, tag="t2b", name="t2b")
                t23 = t2b[0:128].rearrange("p (b n) -> p b n", n=2)
                nc.vector.tensor_mul(t23[:, :, :], t13[:, :, 0:2], t13[:, :, 2:4])
                nc.vector.tensor_mul(
                    t3av[:, :, ch], t23[:, :, 0:1].rearrange("p b n -> p (b n)"),
                    t23[:, :, 1:2].rearrange("p b n -> p (b n)"),
                )
            # the final affine 1 - k_base*t3 is applied host-side
            nc.sync.dma_start(out=out_d[:], in_=t3a[:])

    _split_multiwaits(nc)
    return nc


def kernel(input, time_dis, w_ih, w_hh, b_ih, b_hh, fc2_w, fc2_b, baseline):
    input = np.asarray(input, dtype=np.float32)
    time_dis = np.asarray(time_dis, dtype=np.float32)
    w_ih = np.asarray(w_ih, dtype=np.float32)
    w_hh = np.asarray(w_hh, dtype=np.float32)
    b_ih = np.asarray(b_ih, dtype=np.float32)
    b_hh = np.asarray(b_hh, dtype=np.float32)
    fc2_w = np.asarray(fc2_w, dtype=np.float32)
    fc2_b = np.asarray(fc2_b, dtype=np.float32)
    baseline = np.asarray(baseline, dtype=np.float32)

    bf = ml_dtypes.bfloat16
    bper = BSIZE // NCORES  # 512

    # gates^T = W^T.T @ [x;h], W = [w_ih | w_hh]  [256, 128]
    W = np.concatenate([w_ih, w_hh], axis=1)  # [256, 128]
    lhsT = np.ascontiguousarray(W.T)  # [128, 256] cols: i(0:64) f g o
    li, lf = lhsT[:, 0:64], lhsT[:, 64:128]
    lg, lo = lhsT[:, 128:192], lhsT[:, 192:256]
    wfi = np.concatenate([lf, li], axis=1)
    wif = np.concatenate([li, lf], axis=1)
    fc2col = np.zeros((128, 1), dtype=np.float32)
    fc2col[0:HID, 0] = fc2_w.reshape(HID)
    fc2col[HID:128, 0] = fc2_w.reshape(HID)
    wpack = np.ascontiguousarray(
        np.concatenate([wfi, wif, lg, lo, fc2col], axis=1)
    ).astype(bf)  # [128, 385]
    bias = (b_ih + b_hh).astype(np.float32)
    bi, bfg = bias[0:64], bias[64:128]
    bgg, bog = bias[128:192], bias[192:256]
    bpack = np.ascontiguousarray(
        np.stack(
            [
                np.concatenate([bfg, bi]),
                np.concatenate([bi, bfg]),
                np.concatenate([bgg, bgg]),
                np.concatenate([bog, bog]),
            ],
            axis=1,
        )
    )  # [128, 4] f32
    k_base = float(1.0 - 1.0 / (1.0 + math.exp(-float(baseline[0]))))

    nc = _build(float(fc2_b[0]), k_base)

    in_maps = []
    for k in range(NCORES):
        bs = slice(k * bper, (k + 1) * bper)
        xs = input[:, bs].reshape(STEP, BL, DIM)
        xs = np.ascontiguousarray(xs.transpose(0, 2, 1)).astype(bf)  # [S,64,BL]
        td = time_dis[bs]  # [512, 32]
        td_bn = np.repeat(td.T, NNOD, axis=1)  # [32, 4096] sample-major
        td_used = np.concatenate([td_bn[:1], td_bn[:-1]], axis=0)
        dec = (1.0 / np.log(math.e + td_used)).astype(bf)  # [32, BL]
        # dec2[t, 0:64, j] = dec[t, j] (half0) ; dec2[t, 64:128, j] = dec[t, HALF+j]
        dec2 = np.empty((STEP, 128, HALF), dtype=bf)
        dec2[:, 0:HID, :] = dec[:, None, 0:HALF]
        dec2[:, HID:128, :] = dec[:, None, HALF:BL]
        in_maps.append(
            {
                "x": xs,
                "dec": dec2,
                "wpack": wpack,
                "bpack": bpack,
            }
        )

    res = None
    last_err = None
    for _attempt in range(3):
        try:
            res = run_bass_kernel_spmd(nc, in_maps, list(range(NCORES)))
            break
        except Exception as e:  # transient NRT device errors recover on retry
            last_err = e
    if res is None:
        raise last_err
    global LAST_RESULT
    LAST_RESULT = res
    out = np.concatenate(
        [np.asarray(res.results[k]["out"]).T.reshape(bper) for k in range(NCORES)]
    )
    # device ships t3 = prod_n q; the noisy-OR leak affine runs here
    return (1.0 - k_base * out).astype(np.float32)

